# revision 12
# baseline (speedup 1.0000x reference)
"""Trainium2 Bass kernel for a GPT-style transformer block.

Reference computation (B=4, T=2048, d=1024, 16 heads, dff=4096, fp32):
    h  = LN1(x);  qkv = h @ w_attn + b_attn
    y  = causal_attention(q, k, v);  x1 = x + y @ w_proj + b_proj
    h2 = LN2(x1); out = x1 + gelu(h2 @ w_fc + b_fc) @ w_fc2 + b_fc2

Sharding over 8 NeuronCores (one trn2 chip):
  - Attention is head-parallel: core c owns heads (2c, 2c+1). Each core
    computes LN1 for its own 1024-token slice, the per-core h^T shards are
    AllGathered, each core computes q/k/v (transposed layout) for its two
    heads over all 8192 tokens, runs causal attention for its 8
    (batch, head) units, and the per-core y^T shards are AllGathered.
  - Everything after attention is token-parallel: core c owns flattened
    tokens [1024c, 1024c+1024) and computes proj+residual+LN2+MLP+residual
    for them with full-width weights.

Matmuls run as float32r (full-rate fp32 on the PE when the moving dim is
>= 256); every matmul operand is produced typed float32r (DMA from
f32r-declared DRAM or ACT eviction into an f32r tile) to satisfy the BIR
verifier. Attention uses the transposed-score layout: scores^T[kv, tq] so
the softmax denominator comes free from an appended ones-column in v, and
the causal mask is added (additive -1e9) to PSUM before the exp.
"""

import os
import sys

import numpy as np

sys.path.insert(0, "/opt/trn_rl_repo")

import concourse.bass as bass  # noqa: E402
import concourse.mybir as mybir  # noqa: E402
import concourse.tile as tile  # noqa: E402
from concourse import bacc  # noqa: E402
from concourse.bass_utils import run_bass_kernel_spmd  # noqa: E402
from concourse.masks import make_identity  # noqa: E402

B, T, D, H, HD, DFF = 4, 2048, 1024, 16, 64, 4096
EPS = 1e-5
NCORES = 8
TOK = B * T            # 8192 flattened tokens
TOWN = TOK // NCORES   # 1024 tokens owned per core
P = 128
F32 = mybir.dt.float32
F32R = mybir.dt.float32r
Act = mybir.ActivationFunctionType
Alu = mybir.AluOpType
AX = mybir.AxisListType
NEG = -1.0e9


def build():
    nc = bacc.Bacc("TRN2", target_bir_lowering=False, debug=False, num_devices=NCORES)

    def inp(name, shape, dt=F32):
        return nc.dram_tensor(name, shape, dt, kind="ExternalInput").ap()

    x_own = inp("x_own", [TOWN, D])
    wq = inp("wq", [D, P], F32R)
    wk = inp("wk", [D, P], F32R)
    wv = inp("wv", [D, P], F32R)
    bq = inp("bq", [P, 1])
    bk = inp("bk", [P, 1])
    bv = inp("bv", [P, 1])
    ln1w = inp("ln1w", [P, 8])
    ln1b = inp("ln1b", [P, 8])
    ln2w = inp("ln2w", [P, 8])
    ln2b = inp("ln2b", [P, 8])
    wproj = inp("wproj", [D, D], F32R)
    bproj = inp("bproj", [1, D], F32R)
    wfc = inp("wfc", [D, DFF], F32R)
    bfc = inp("bfc", [P, DFF // P])
    wfc2 = inp("wfc2", [DFF, D], F32R)
    bfc2 = inp("bfc2", [1, D], F32R)
    out_own = nc.dram_tensor("out", [TOWN, D], F32, kind="ExternalOutput").ap()

    groups = [list(range(NCORES))]

    with tile.TileContext(nc) as tc:
        with (
            tc.tile_pool(name="const", bufs=1) as cst,
            tc.tile_pool(name="dram", bufs=1, space="DRAM") as dram,
        ):
            # ---------------- constants ----------------
            ident = cst.tile([P, P], F32)
            make_identity(nc, ident)
            ones_f = cst.tile([1, P], F32)
            nc.vector.memset(ones_f[:], 1.0)
            ones_r = cst.tile([1, P], F32R)
            nc.scalar.copy(ones_r[:], ones_f[:])
            ones_col = cst.tile([P, 16], F32)
            nc.vector.memset(ones_col[:], 1.0)
            ln1w_sb = cst.tile([P, 8], F32)
            nc.sync.dma_start(ln1w_sb[:], ln1w)
            ln1b_sb = cst.tile([P, 8], F32)
            nc.sync.dma_start(ln1b_sb[:], ln1b)
            ln2w_sb = cst.tile([P, 8], F32)
            nc.sync.dma_start(ln2w_sb[:], ln2w)
            ln2b_sb = cst.tile([P, 8], F32)
            nc.sync.dma_start(ln2b_sb[:], ln2b)
            bq_sb = cst.tile([P, 1], F32)
            nc.sync.dma_start(bq_sb[:], bq)
            bk_sb = cst.tile([P, 1], F32)
            nc.sync.dma_start(bk_sb[:], bk)
            bv_sb = cst.tile([P, 1], F32)
            nc.sync.dma_start(bv_sb[:], bv)
            bproj_sb = cst.tile([1, D], F32R)
            nc.sync.dma_start(bproj_sb[:], bproj)
            bfc_sb = cst.tile([P, DFF // P], F32)
            nc.sync.dma_start(bfc_sb[:], bfc)
            bfc2_sb = cst.tile([1, D], F32R)
            nc.sync.dma_start(bfc2_sb[:], bfc2)
            # additive causal masks for the 4 diagonal offsets:
            # maskadd[s][i, j] = 0 if i <= j - 128*s else -1e9
            maskadd = cst.tile([P, 4, 512], F32)
            nc.vector.memset(maskadd[:], 0.0)
            for s in range(4):
                nc.gpsimd.affine_select(
                    out=maskadd[:, s, :],
                    in_=maskadd[:, s, :],
                    pattern=[[1, 512]],
                    channel_multiplier=-1,
                    base=-128 * s,
                    compare_op=Alu.is_ge,
                    fill=NEG,
                )

            # DRAM intermediates: h^T shard (AllGathered to all cores) and
            # y^T chunks (AllToAll: head-sharded -> token-sharded)
            hT_dram = dram.tile([D, TOWN], F32R)
            hT_full = dram.tile([NCORES * D, TOWN], F32R, addr_space="Shared")
            yT_send = dram.tile([NCORES, P, TOWN], F32R)
            yT_recv = dram.tile([NCORES, P, TOWN], F32R)

            # =========================================================
            # Phase 1: LN1 over own tokens -> h^T shard -> AllGather
            # =========================================================
            def layernorm_tile(pool, xt, w_sb, b_sb, ps_pool, dstT, t):
                """LN a [128, D] token tile and write transposed blocks
                (with gamma/beta applied) into dstT[:, dblk, t, :] (f32r)."""
                ssum = pool.tile([P, 1], F32, tag="ssum")
                nc.vector.reduce_sum(ssum[:], xt[:], axis=AX.X)
                mean = pool.tile([P, 1], F32, tag="mean")
                nc.scalar.mul(mean[:], ssum[:], 1.0 / D)
                sq = pool.tile([P, D], F32, tag="sq")
                sumsq = pool.tile([P, 1], F32, tag="sumsq")
                nc.scalar.activation(sq[:], xt[:], Act.Square, accum_out=sumsq[:])
                msq = pool.tile([P, 1], F32, tag="msq")
                nc.vector.tensor_tensor(msq[:], mean[:], mean[:], op=Alu.mult)
                var = pool.tile([P, 1], F32, tag="var")
                nc.vector.tensor_scalar(var[:], sumsq[:], 1.0 / D, EPS, Alu.mult, Alu.add)
                nc.vector.tensor_tensor(var[:], var[:], msq[:], op=Alu.subtract)
                rinv = pool.tile([P, 1], F32, tag="rinv")
                nc.vector.reciprocal(rinv[:], var[:])
                rstd = pool.tile([P, 1], F32, tag="rstd")
                nc.scalar.sqrt(rstd[:], rinv[:])
                hh = pool.tile([P, D], F32, tag="hh")
                nc.vector.tensor_scalar(
                    hh[:], xt[:], mean[:], rstd[:], Alu.subtract, Alu.mult
                )
                for dblk in range(8):
                    pt = ps_pool.tile([P, P], F32, tag="lnt")
                    nc.tensor.transpose(pt[:], hh[:, dblk * P : (dblk + 1) * P], ident[:])
                    nc.scalar.activation(
                        dstT[:, dblk, t, :],
                        pt[:],
                        Act.Identity,
                        bias=b_sb[:, dblk : dblk + 1],
                        scale=w_sb[:, dblk : dblk + 1],
                    )

            with (
                tc.tile_pool(name="ph1", bufs=2) as ph1,
                tc.tile_pool(name="ph1T", bufs=1) as ph1T,
                tc.tile_pool(name="psA", bufs=2, space="PSUM") as psA,
            ):
                hT_asm = ph1T.tile([P, 8, 8, P], F32R)  # [p, dblk, t, j]
                for t in range(8):
                    xt = ph1.tile([P, D], F32, tag="xt")
                    nc.sync.dma_start(xt[:], x_own[t * P : (t + 1) * P, :])
                    layernorm_tile(ph1, xt, ln1w_sb, ln1b_sb, psA, hT_asm, t)
                hTv = hT_dram.rearrange("(dblk p) t -> p dblk t", p=P)
                for dblk in range(8):
                    nc.sync.dma_start(hTv[:, dblk, :], hT_asm[:, dblk, :, :])

            nc.gpsimd.collective_compute(
                "AllGather", Alu.bypass, replica_groups=groups,
                ins=[hT_dram[:]], outs=[hT_full[:]],
            )

            # =========================================================
            # Phase 2: q^T/k^T/v^T for this core's two heads, all tokens
            # =========================================================
            # token index = tile16 * 512 + j
            with tc.tile_pool(name="qkv", bufs=1) as qkvp:
                qT = qkvp.tile([P, 16, 512], F32R)
                kT = qkvp.tile([P, 16, 512], F32R)
                vT = qkvp.tile([P, 16, 512], F32R)
                with (
                    tc.tile_pool(name="wqkv", bufs=1) as wp,
                    tc.tile_pool(name="ph2", bufs=2) as ph2,
                    tc.tile_pool(name="psB", bufs=3, space="PSUM") as psB,
                ):
                    wq_sb = wp.tile([P, 8, P], F32R)
                    nc.sync.dma_start(wq_sb[:], wq.rearrange("(ko p) m -> p ko m", p=P))
                    wk_sb = wp.tile([P, 8, P], F32R)
                    nc.sync.dma_start(wk_sb[:], wk.rearrange("(ko p) m -> p ko m", p=P))
                    wv_sb = wp.tile([P, 8, P], F32R)
                    nc.sync.dma_start(wv_sb[:], wv.rearrange("(ko p) m -> p ko m", p=P))
                    hfv = hT_full.rearrange("(r ko p) t -> r p ko t", p=P, ko=8)
                    for rr in range(8):
                        for n in range(2):
                            ht = ph2.tile([P, 8, 512], F32R, tag="ht")
                            nc.sync.dma_start(
                                ht[:], hfv[rr, :, :, n * 512 : (n + 1) * 512]
                            )
                            for w_sb, b_sb, dstT in (
                                (wq_sb, bq_sb, qT),
                                (wk_sb, bk_sb, kT),
                                (wv_sb, bv_sb, vT),
                            ):
                                ps = psB.tile([P, 512], F32, tag="qkvps")
                                for ko in range(8):
                                    nc.tensor.matmul(
                                        ps[:], w_sb[:, ko, :], ht[:, ko, :],
                                        start=(ko == 0), stop=(ko == 7),
                                    )
                                nc.scalar.activation(
                                    dstT[:, rr * 2 + n, :], ps[:], Act.Identity,
                                    bias=b_sb[:],
                                )

                # =====================================================
                # Phase 3: causal attention for 8 (batch, head) units
                # =====================================================
                with (
                    tc.tile_pool(name="ph3", bufs=3) as ph3,
                    tc.tile_pool(name="vnp", bufs=2) as vnp,
                    tc.tile_pool(name="psQK", bufs=2, space="PSUM") as psQK,
                    tc.tile_pool(name="psY", bufs=2, space="PSUM") as psY,
                    tc.tile_pool(name="psT", bufs=2, space="PSUM") as psT,
                ):
                    for b in range(B):
                        for h in range(2):
                            hp = h * HD  # partition offset of this head
                            # v natural [kv, hd] with an appended ones column
                            vn = vnp.tile([P, 16, HD + 1], F32R, tag="vn")
                            nc.scalar.copy(vn[:, :, HD], ones_col[:])
                            for kb in range(16):
                                pt = psT.tile([P, HD], F32, tag="vtp")
                                nc.tensor.transpose(
                                    pt[:],
                                    vT[hp : hp + HD, b * 4 + kb // 4,
                                       (kb % 4) * P : (kb % 4 + 1) * P].bitcast(F32),
                                    ident[hp : hp + HD, hp : hp + HD],
                                )
                                nc.scalar.copy(vn[:, kb, 0:HD], pt[:])
                            for tq in range(4):
                                nkv = 4 * (tq + 1)
                                py = psY.tile([HD + 1, 512], F32, tag="py")
                                pend = []  # pipelined AV emission
                                for kb in range(nkv):
                                    ps = psQK.tile([P, 512], F32, tag="qk")
                                    nc.tensor.matmul(
                                        ps[:],
                                        kT[hp : hp + HD, b * 4 + kb // 4,
                                           (kb % 4) * P : (kb % 4 + 1) * P],
                                        qT[hp : hp + HD, b * 4 + tq, :],
                                        start=True, stop=True,
                                    )
                                    if kb >= 4 * tq:
                                        nc.vector.tensor_tensor(
                                            ps[:], ps[:], maskadd[:, kb - 4 * tq, :],
                                            op=Alu.add,
                                        )
                                    ex = ph3.tile([P, 512], F32R, tag="ex")
                                    nc.scalar.activation(
                                        ex[:], ps[:], Act.Exp, scale=1.0 / np.sqrt(HD)
                                    )
                                    pend.append((kb, ex))
                                    if len(pend) > 1:  # keep one QK ahead of AV
                                        k0, e0 = pend.pop(0)
                                        nc.tensor.matmul(
                                            py[:], vn[:, k0, :], e0[:],
                                            start=(k0 == 0), stop=False,
                                        )
                                k0, e0 = pend.pop(0)
                                nc.tensor.matmul(
                                    py[:], vn[:, k0, :], e0[:],
                                    start=(k0 == 0), stop=True,
                                )
                                # normalize by the ones-column sums (row HD)
                                sums = ph3.tile([1, 512], F32, tag="sums")
                                nc.scalar.copy(sums[:], py[HD : HD + 1, :])
                                rec = ph3.tile([1, 512], F32, tag="rec")
                                nc.vector.reciprocal(rec[:], sums[:])
                                rec_r = ph3.tile([1, 512], F32R, tag="recr")
                                nc.scalar.copy(rec_r[:], rec[:])
                                pb = psT.tile([HD, 512], F32, tag="pb", bufs=1)
                                nc.tensor.matmul(
                                    pb[:], ones_r[:, :HD], rec_r[:],
                                    start=True, stop=True,
                                )
                                pb_sb = ph3.tile([HD, 512], F32, tag="pbs")
                                nc.scalar.copy(pb_sb[:], pb[:])
                                yt = ph3.tile([HD, 512], F32, tag="yt")
                                nc.vector.tensor_tensor(
                                    yt[:], py[0:HD, :], pb_sb[:], op=Alu.mult
                                )
                                # global token start of this tq tile
                                t0 = b * T + tq * 512
                                nc.sync.dma_start(
                                    yT_send[t0 // TOWN, hp : hp + HD,
                                            t0 % TOWN : t0 % TOWN + 512],
                                    yt[:].bitcast(F32R),
                                )

            # head-sharded y^T -> token-sharded y^T (chunk j goes to core j)
            nc.gpsimd.collective_compute(
                "AllToAll", Alu.bypass, replica_groups=groups,
                ins=[yT_send[:]], outs=[yT_recv[:]],
            )

            # =========================================================
            # Phase 4: proj + residual + LN2 (own tokens) -> h2^T (SBUF)
            # =========================================================
            with tc.tile_pool(name="keep", bufs=1) as keep:
                h2T = keep.tile([P, 8, 8, P], F32R)  # [p, dblk, t, j]
                x1_sb = keep.tile([P, 8, D], F32)    # [p, t, d]
                with (
                    tc.tile_pool(name="ph4", bufs=2) as ph4,
                    tc.tile_pool(name="wpp", bufs=1) as wpp,
                    tc.tile_pool(name="psC", bufs=3, space="PSUM") as psC,
                    tc.tile_pool(name="psD", bufs=2, space="PSUM") as psD,
                ):
                    wproj_sb = wpp.tile([P, 8, D], F32R)
                    nc.sync.dma_start(
                        wproj_sb[:], wproj.rearrange("(ko p) n -> p ko n", p=P)
                    )
                    # yT_recv[ko, p, t]: d = 128*ko + p, t = own local token
                    yfv = yT_recv.rearrange("ko p t -> p ko t")
                    for t in range(8):
                        yt_own = ph4.tile([P, 8, P], F32R, tag="ytown")
                        nc.sync.dma_start(
                            yt_own[:], yfv[:, :, t * P : (t + 1) * P]
                        )
                        xt = ph4.tile([P, D], F32, tag="xt4")
                        nc.sync.dma_start(xt[:], x_own[t * P : (t + 1) * P, :])
                        for n in range(2):
                            pp = psC.tile([P, 512], F32, tag="pj")
                            nc.tensor.matmul(
                                pp[:], ones_r[:, :P],
                                bproj_sb[:, n * 512 : (n + 1) * 512],
                                start=True, stop=False,
                            )
                            for ko in range(8):
                                nc.tensor.matmul(
                                    pp[:], yt_own[:, ko, :],
                                    wproj_sb[:, ko, n * 512 : (n + 1) * 512],
                                    start=False, stop=(ko == 7),
                                )
                            nc.vector.tensor_tensor(
                                x1_sb[:, t, n * 512 : (n + 1) * 512], pp[:],
                                xt[:, n * 512 : (n + 1) * 512], op=Alu.add,
                            )
                        layernorm_tile(
                            ph4, x1_sb[:, t, :], ln2w_sb, ln2b_sb, psD, h2T, t
                        )

                # =====================================================
                # Phase 5: MLP (own tokens, 2 groups of 512)
                # =====================================================
                with (
                    tc.tile_pool(name="mt", bufs=1) as mtp,
                    tc.tile_pool(name="ph5", bufs=3) as ph5,
                    tc.tile_pool(name="psM", bufs=2, space="PSUM") as psM,
                    tc.tile_pool(name="psO", bufs=1, space="PSUM") as psO,
                ):
                    wfcv = wfc.rearrange("(ko p) n -> p ko n", p=P)
                    wfc2v = wfc2.rearrange("(kb p) n -> p kb n", p=P)
                    for g in range(2):
                        mT = mtp.tile([P, DFF // P, 512], F32R, tag="mt")
                        for kb in range(DFF // P):
                            wt = ph5.tile([P, 8, P], F32R, tag="wfct")
                            nc.sync.dma_start(
                                wt[:], wfcv[:, :, kb * P : (kb + 1) * P]
                            )
                            pm = psM.tile([P, 512], F32, tag="pm")
                            for ko in range(8):
                                nc.tensor.matmul(
                                    pm[:], wt[:, ko, :],
                                    h2T[:, ko, 4 * g : 4 * g + 4, :],
                                    start=(ko == 0), stop=(ko == 7),
                                )
                            nc.scalar.activation(
                                mT[:, kb, :], pm[:], Act.Gelu,
                                bias=bfc_sb[:, kb : kb + 1],
                            )
                        for n2 in range(2):
                            pos = [
                                psO.tile([P, 512], F32, tag=f"po{t2}",
                                         name=f"po_{g}_{n2}_{t2}")
                                for t2 in range(4)
                            ]
                            for t2 in range(4):
                                nc.tensor.matmul(
                                    pos[t2][:], ones_r[:, :P],
                                    bfc2_sb[:, n2 * 512 : (n2 + 1) * 512],
                                    start=True, stop=False,
                                )
                            for kb in range(DFF // P):
                                w2 = ph5.tile([P, 512], F32R, tag="wfc2t")
                                nc.sync.dma_start(
                                    w2[:], wfc2v[:, kb, n2 * 512 : (n2 + 1) * 512]
                                )
                                for t2 in range(4):
                                    nc.tensor.matmul(
                                        pos[t2][:], mT[:, kb, t2 * P : (t2 + 1) * P],
                                        w2[:],
                                        start=False, stop=(kb == DFF // P - 1),
                                    )
                            for t2 in range(4):
                                t = 4 * g + t2
                                ot = ph5.tile([P, 512], F32, tag="ot")
                                nc.vector.tensor_tensor(
                                    ot[:], pos[t2][:],
                                    x1_sb[:, t, n2 * 512 : (n2 + 1) * 512],
                                    op=Alu.add,
                                )
                                nc.sync.dma_start(
                                    out_own[t * P : (t + 1) * P,
                                            n2 * 512 : (n2 + 1) * 512],
                                    ot[:],
                                )

    nc.compile()
    return nc


_NC_CACHE = None


def kernel(x, ln1_w, ln1_b, ln2_w, ln2_b, w_attn, b_attn, w_proj, b_proj,
           w_fc, b_fc, w_fc2, b_fc2):
    global _NC_CACHE
    if _NC_CACHE is None:
        _NC_CACHE = build()
    nc = _NC_CACHE

    f = np.ascontiguousarray
    x = np.asarray(x, np.float32)
    xf = x.reshape(TOK, D)
    w_attn = np.asarray(w_attn, np.float32)
    b_attn = np.asarray(b_attn, np.float32)

    def col(v, c):  # [128, 1] bias slice
        return f(np.asarray(v, np.float32)[c * P : (c + 1) * P].reshape(P, 1))

    def strip(v):  # [1024] -> [128, 8] with [p, a] = v[a*128 + p]
        return f(np.asarray(v, np.float32).reshape(-1, P).T)

    in_maps = []
    for c in range(NCORES):
        in_maps.append({
            "x_own": f(xf[c * TOWN : (c + 1) * TOWN]),
            "wq": f(w_attn[:, P * c : P * (c + 1)]),
            "wk": f(w_attn[:, D + P * c : D + P * (c + 1)]),
            "wv": f(w_attn[:, 2 * D + P * c : 2 * D + P * (c + 1)]),
            "bq": col(b_attn, c),
            "bk": col(b_attn[D:], c),
            "bv": col(b_attn[2 * D:], c),
            "ln1w": strip(ln1_w), "ln1b": strip(ln1_b),
            "ln2w": strip(ln2_w), "ln2b": strip(ln2_b),
            "wproj": f(np.asarray(w_proj, np.float32)),
            "bproj": f(np.asarray(b_proj, np.float32).reshape(1, D)),
            "wfc": f(np.asarray(w_fc, np.float32)),
            "bfc": strip(b_fc),
            "wfc2": f(np.asarray(w_fc2, np.float32)),
            "bfc2": f(np.asarray(b_fc2, np.float32).reshape(1, D)),
        })

    global _last_in_maps
    _last_in_maps = in_maps
    res = run_bass_kernel_spmd(nc, in_maps, core_ids=list(range(NCORES)))
    out = np.concatenate([res.results[c]["out"] for c in range(NCORES)], axis=0)
    return out.reshape(B, T, D)


_last_in_maps = None


# revision 14
# speedup vs baseline: 1.1875x; 1.1875x over previous
"""Trainium2 Bass kernel for a GPT-style transformer block.

Reference computation (B=4, T=2048, d=1024, 16 heads, dff=4096, fp32):
    h  = LN1(x);  qkv = h @ w_attn + b_attn
    y  = causal_attention(q, k, v);  x1 = x + y @ w_proj + b_proj
    h2 = LN2(x1); out = x1 + gelu(h2 @ w_fc + b_fc) @ w_fc2 + b_fc2

Sharding over 8 NeuronCores (one trn2 chip):
  - Attention is head-parallel: core c owns heads (2c, 2c+1). Each core
    computes LN1 for its own 1024-token slice, the per-core h^T shards are
    AllGathered (bf16), each core computes q/k/v (transposed layout) for
    its two heads over all 8192 tokens, and runs causal attention for its
    8 (batch, head) units. The per-core y^T shards are redistributed with
    an AllToAll into token-sharded form.
  - Everything after attention is token-parallel: core c owns flattened
    tokens [1024c, 1024c+1024) and computes proj+residual+LN2+MLP+residual
    for them with full-width weights.

Precision: the residual stream, layernorm statistics, attention scores
(QK in float32r = full-rate fp32) and softmax sums stay fp32-ish; the
h-shard, v/attention-weights and the MLP run in bf16 (all accumulation in
fp32 PSUM). Measured end-to-end relative error ~1e-3 against the fp32
reference. Attention uses the transposed-score layout: scores^T[kv, tq]
so the softmax denominator comes free from an appended ones-column in v,
and the causal mask is added (additive -1e9) to PSUM before the exp.
"""

import os
import sys

import numpy as np
import ml_dtypes

sys.path.insert(0, "/opt/trn_rl_repo")

import concourse.bass as bass  # noqa: E402
import concourse.mybir as mybir  # noqa: E402
import concourse.tile as tile  # noqa: E402
from concourse import bacc  # noqa: E402
from concourse.bass_utils import run_bass_kernel_spmd  # noqa: E402
from concourse.masks import make_identity  # noqa: E402

B, T, D, H, HD, DFF = 4, 2048, 1024, 16, 64, 4096
EPS = 1e-5
NCORES = 8
TOK = B * T            # 8192 flattened tokens
TOWN = TOK // NCORES   # 1024 tokens owned per core
P = 128
F32 = mybir.dt.float32
F32R = mybir.dt.float32r
BF16 = mybir.dt.bfloat16
Act = mybir.ActivationFunctionType
Alu = mybir.AluOpType
AX = mybir.AxisListType
NEG = -1.0e9
BF = ml_dtypes.bfloat16


def build():
    nc = bacc.Bacc("TRN2", target_bir_lowering=False, debug=False, num_devices=NCORES)

    def inp(name, shape, dt=F32):
        return nc.dram_tensor(name, shape, dt, kind="ExternalInput").ap()

    x_own = inp("x_own", [TOWN, D])
    wq = inp("wq", [D, P], BF16)
    wk = inp("wk", [D, P], BF16)
    wv = inp("wv", [D, P], BF16)
    bq = inp("bq", [P, 1])
    bk = inp("bk", [P, 1])
    bv = inp("bv", [P, 1])
    ln1w = inp("ln1w", [P, 8])
    ln1b = inp("ln1b", [P, 8])
    ln2w = inp("ln2w", [P, 8])
    ln2b = inp("ln2b", [P, 8])
    wproj = inp("wproj", [D, D], F32R)
    bproj = inp("bproj", [1, D], F32R)
    wfc = inp("wfc", [D, DFF], BF16)
    bfc = inp("bfc", [P, DFF // P])
    wfc2 = inp("wfc2", [DFF, D], BF16)
    bfc2 = inp("bfc2", [1, D], BF16)
    out_own = nc.dram_tensor("out", [TOWN, D], F32, kind="ExternalOutput").ap()

    groups = [list(range(NCORES))]

    with tile.TileContext(nc) as tc:
        with (
            tc.tile_pool(name="const", bufs=1) as cst,
            tc.tile_pool(name="dram", bufs=1, space="DRAM") as dram,
        ):
            # ---------------- constants ----------------
            ident = cst.tile([P, P], F32)
            make_identity(nc, ident)
            ident_bf = cst.tile([P, P], BF16)
            make_identity(nc, ident_bf)
            ones_f = cst.tile([1, P], F32)
            nc.vector.memset(ones_f[:], 1.0)
            ones_r = cst.tile([1, P], F32R)
            nc.scalar.copy(ones_r[:], ones_f[:])
            ones_b = cst.tile([1, P], BF16)
            nc.scalar.copy(ones_b[:], ones_f[:])
            ln1w_sb = cst.tile([P, 8], F32)
            nc.sync.dma_start(ln1w_sb[:], ln1w)
            ln1b_sb = cst.tile([P, 8], F32)
            nc.sync.dma_start(ln1b_sb[:], ln1b)
            ln2w_sb = cst.tile([P, 8], F32)
            nc.sync.dma_start(ln2w_sb[:], ln2w)
            ln2b_sb = cst.tile([P, 8], F32)
            nc.sync.dma_start(ln2b_sb[:], ln2b)
            bq_sb = cst.tile([P, 1], F32)
            nc.sync.dma_start(bq_sb[:], bq)
            bk_sb = cst.tile([P, 1], F32)
            nc.sync.dma_start(bk_sb[:], bk)
            bv_sb = cst.tile([P, 1], F32)
            nc.sync.dma_start(bv_sb[:], bv)
            bproj_sb = cst.tile([1, D], F32R)
            nc.sync.dma_start(bproj_sb[:], bproj)
            bfc_sb = cst.tile([P, DFF // P], F32)
            nc.sync.dma_start(bfc_sb[:], bfc)
            bfc2_sb = cst.tile([1, D], BF16)
            nc.sync.dma_start(bfc2_sb[:], bfc2)
            # additive causal masks for the 4 diagonal offsets:
            # maskadd[s][i, j] = 0 if i <= j - 128*s else -1e9
            maskadd = cst.tile([P, 4, 512], F32)
            nc.vector.memset(maskadd[:], 0.0)
            for s in range(4):
                nc.gpsimd.affine_select(
                    out=maskadd[:, s, :],
                    in_=maskadd[:, s, :],
                    pattern=[[1, 512]],
                    channel_multiplier=-1,
                    base=-128 * s,
                    compare_op=Alu.is_ge,
                    fill=NEG,
                )

            # DRAM intermediates: h^T shard (AllGathered to all cores, bf16)
            # and y^T chunks (AllToAll: head-sharded -> token-sharded, f32r)
            hT_dram = dram.tile([D, TOWN], BF16)
            hT_full = dram.tile([NCORES * D, TOWN], BF16, addr_space="Shared")
            yT_send = dram.tile([NCORES, P, TOWN], F32R)
            yT_recv = dram.tile([NCORES, P, TOWN], F32R)

            # =========================================================
            # Phase 1: LN1 over own tokens -> h^T shard -> AllGather
            # =========================================================
            def layernorm_tile(pool, xt, w_sb, b_sb, ps_pool, dstT, t):
                """LN a [128, D] token tile and write transposed blocks
                (with gamma/beta applied) into dstT[:, dblk, t, :] (bf16)."""
                ssum = pool.tile([P, 1], F32, tag="ssum")
                nc.vector.reduce_sum(ssum[:], xt[:], axis=AX.X)
                mean = pool.tile([P, 1], F32, tag="mean")
                nc.scalar.mul(mean[:], ssum[:], 1.0 / D)
                sq = pool.tile([P, D], F32, tag="sq")
                sumsq = pool.tile([P, 1], F32, tag="sumsq")
                nc.scalar.activation(sq[:], xt[:], Act.Square, accum_out=sumsq[:])
                msq = pool.tile([P, 1], F32, tag="msq")
                nc.vector.tensor_tensor(msq[:], mean[:], mean[:], op=Alu.mult)
                var = pool.tile([P, 1], F32, tag="var")
                nc.vector.tensor_scalar(var[:], sumsq[:], 1.0 / D, EPS, Alu.mult, Alu.add)
                nc.vector.tensor_tensor(var[:], var[:], msq[:], op=Alu.subtract)
                rinv = pool.tile([P, 1], F32, tag="rinv")
                nc.vector.reciprocal(rinv[:], var[:])
                rstd = pool.tile([P, 1], F32, tag="rstd")
                nc.scalar.sqrt(rstd[:], rinv[:])
                hh = pool.tile([P, D], F32, tag="hh")
                nc.vector.tensor_scalar(
                    hh[:], xt[:], mean[:], rstd[:], Alu.subtract, Alu.mult
                )
                for dblk in range(8):
                    pt = ps_pool.tile([P, P], F32, tag="lnt")
                    nc.tensor.transpose(pt[:], hh[:, dblk * P : (dblk + 1) * P], ident[:])
                    nc.scalar.activation(
                        dstT[:, dblk, t, :],
                        pt[:],
                        Act.Identity,
                        bias=b_sb[:, dblk : dblk + 1],
                        scale=w_sb[:, dblk : dblk + 1],
                    )

            with (
                tc.tile_pool(name="ph1", bufs=2) as ph1,
                tc.tile_pool(name="ph1T", bufs=1) as ph1T,
                tc.tile_pool(name="psA", bufs=2, space="PSUM") as psA,
                nc.named_scope("ph1_ln1"),
            ):
                hT_asm = ph1T.tile([P, 8, 8, P], BF16)  # [p, dblk, t, j]
                for t in range(8):
                    xt = ph1.tile([P, D], F32, tag="xt")
                    nc.sync.dma_start(xt[:], x_own[t * P : (t + 1) * P, :])
                    layernorm_tile(ph1, xt, ln1w_sb, ln1b_sb, psA, hT_asm, t)
                hTv = hT_dram.rearrange("(dblk p) t -> p dblk t", p=P)
                for dblk in range(8):
                    nc.sync.dma_start(hTv[:, dblk, :], hT_asm[:, dblk, :, :])

            with nc.named_scope("cc_ag_h"):
                nc.gpsimd.collective_compute(
                    "AllGather", Alu.bypass, replica_groups=groups,
                    ins=[hT_dram[:]], outs=[hT_full[:]],
                )

            # =========================================================
            # Phase 2: q^T/k^T (f32r) and v^T (bf16) for this core's two
            # heads over all tokens; token index = tile16 * 512 + j
            # =========================================================
            with tc.tile_pool(name="qkv", bufs=1) as qkvp:
                qT = qkvp.tile([P, 16, 512], F32R)
                kT = qkvp.tile([P, 16, 512], F32R)
                vT = qkvp.tile([P, 16, 512], BF16)
                with (
                    tc.tile_pool(name="wqkv", bufs=1) as wp,
                    tc.tile_pool(name="ph2", bufs=3) as ph2,
                    tc.tile_pool(name="psB", bufs=3, space="PSUM") as psB,
                    nc.named_scope("ph2_qkv"),
                ):
                    wq_sb = wp.tile([P, 8, P], BF16)
                    nc.sync.dma_start(wq_sb[:], wq.rearrange("(ko p) m -> p ko m", p=P))
                    wk_sb = wp.tile([P, 8, P], BF16)
                    nc.sync.dma_start(wk_sb[:], wk.rearrange("(ko p) m -> p ko m", p=P))
                    wv_sb = wp.tile([P, 8, P], BF16)
                    nc.sync.dma_start(wv_sb[:], wv.rearrange("(ko p) m -> p ko m", p=P))
                    hfv = hT_full.rearrange("(r ko p) t -> r p ko t", p=P, ko=8)
                    for rr in range(8):
                        for n in range(2):
                            ht = ph2.tile([P, 8, 512], BF16, tag="ht")
                            nc.sync.dma_start(
                                ht[:], hfv[rr, :, :, n * 512 : (n + 1) * 512]
                            )
                            for w_sb, b_sb, dstT in (
                                (wq_sb, bq_sb, qT),
                                (wk_sb, bk_sb, kT),
                                (wv_sb, bv_sb, vT),
                            ):
                                ps = psB.tile([P, 512], F32, tag="qkvps")
                                for ko in range(8):
                                    nc.tensor.matmul(
                                        ps[:], w_sb[:, ko, :], ht[:, ko, :],
                                        start=(ko == 0), stop=(ko == 7),
                                    )
                                nc.scalar.activation(
                                    dstT[:, rr * 2 + n, :], ps[:], Act.Identity,
                                    bias=b_sb[:],
                                )

                # =====================================================
                # Phase 3: causal attention for 8 (batch, head) units
                # =====================================================
                with (
                    tc.tile_pool(name="ph3", bufs=4) as ph3,
                    tc.tile_pool(name="ph3s", bufs=3) as ph3s,
                    tc.tile_pool(name="vnp", bufs=2) as vnp,
                    tc.tile_pool(name="psQK", bufs=3, space="PSUM") as psQK,
                    tc.tile_pool(name="psY", bufs=2, space="PSUM") as psY,
                    tc.tile_pool(name="psT", bufs=2, space="PSUM") as psT,
                    nc.named_scope("ph3_attn"),
                ):
                    for b in range(B):
                        for h in range(2):
                            hp = h * HD  # partition offset of this head
                            # v natural [kv, hd] with an appended ones column
                            vn = vnp.tile([P, 16, HD + 1], BF16, tag="vn")
                            nc.scalar.copy(vn[:, :, HD], ones_col_bf(cst, nc))
                            for kb in range(16):
                                pt = psT.tile([P, HD], BF16, tag="vtp")
                                nc.tensor.transpose(
                                    pt[:],
                                    vT[hp : hp + HD, b * 4 + kb // 4,
                                       (kb % 4) * P : (kb % 4 + 1) * P],
                                    ident_bf[hp : hp + HD, hp : hp + HD],
                                )
                                nc.vector.tensor_copy(vn[:, kb, 0:HD], pt[:])
                            for tq in range(4):
                                nkv = 4 * (tq + 1)
                                py = psY.tile([HD + 1, 512], F32, tag="py")
                                pend = []  # pipelined AV emission, lag 2
                                for kb in range(nkv):
                                    ps = psQK.tile([P, 512], F32, tag="qk")
                                    nc.tensor.matmul(
                                        ps[:],
                                        kT[hp : hp + HD, b * 4 + kb // 4,
                                           (kb % 4) * P : (kb % 4 + 1) * P],
                                        qT[hp : hp + HD, b * 4 + tq, :],
                                        start=True, stop=True,
                                    )
                                    if kb >= 4 * tq:
                                        nc.vector.tensor_tensor(
                                            ps[:], ps[:], maskadd[:, kb - 4 * tq, :],
                                            op=Alu.add,
                                        )
                                    ex = ph3.tile([P, 512], BF16, tag="ex")
                                    nc.scalar.activation(
                                        ex[:], ps[:], Act.Exp, scale=1.0 / np.sqrt(HD)
                                    )
                                    pend.append((kb, ex))
                                    if len(pend) > 2:  # keep two QKs ahead of AV
                                        k0, e0 = pend.pop(0)
                                        nc.tensor.matmul(
                                            py[:], vn[:, k0, :], e0[:],
                                            start=(k0 == 0), stop=False,
                                        )
                                while pend:
                                    k0, e0 = pend.pop(0)
                                    nc.tensor.matmul(
                                        py[:], vn[:, k0, :], e0[:],
                                        start=(k0 == 0), stop=(not pend),
                                    )
                                # normalize by the ones-column sums (row HD)
                                rec = ph3s.tile([1, 512], F32, tag="rec")
                                nc.vector.reciprocal(rec[:], py[HD : HD + 1, :])
                                pbc = ph3s.tile([HD, 512], F32, tag="pbc")
                                nc.gpsimd.partition_broadcast(pbc[:], rec[:])
                                yt = ph3s.tile([HD, 512], F32, tag="yt")
                                nc.vector.tensor_tensor(
                                    yt[:], py[0:HD, :], pbc[:], op=Alu.mult
                                )
                                # global token start of this tq tile
                                t0 = b * T + tq * 512
                                nc.sync.dma_start(
                                    yT_send[t0 // TOWN, hp : hp + HD,
                                            t0 % TOWN : t0 % TOWN + 512],
                                    yt[:].bitcast(F32R),
                                )

            # head-sharded y^T -> token-sharded y^T (chunk j goes to core j)
            with nc.named_scope("cc_a2a_y"):
                nc.gpsimd.collective_compute(
                    "AllToAll", Alu.bypass, replica_groups=groups,
                    ins=[yT_send[:]], outs=[yT_recv[:]],
                )

            # =========================================================
            # Phase 4: proj + residual + LN2 (own tokens) -> h2^T (SBUF)
            # =========================================================
            with tc.tile_pool(name="keep", bufs=1) as keep:
                h2T = keep.tile([P, 8, 8, P], BF16)  # [p, dblk, t, j]
                x1_sb = keep.tile([P, 8, D], F32)    # [p, t, d]
                with (
                    tc.tile_pool(name="ph4", bufs=2) as ph4,
                    tc.tile_pool(name="wpp", bufs=1) as wpp,
                    tc.tile_pool(name="psC", bufs=3, space="PSUM") as psC,
                    tc.tile_pool(name="psD", bufs=2, space="PSUM") as psD,
                    nc.named_scope("ph4_proj_ln2"),
                ):
                    wproj_sb = wpp.tile([P, 8, D], F32R)
                    nc.sync.dma_start(
                        wproj_sb[:], wproj.rearrange("(ko p) n -> p ko n", p=P)
                    )
                    # yT_recv[ko, p, t]: d = 128*ko + p, t = own local token
                    yfv = yT_recv.rearrange("ko p t -> p ko t")
                    for t in range(8):
                        yt_own = ph4.tile([P, 8, P], F32R, tag="ytown")
                        nc.sync.dma_start(
                            yt_own[:], yfv[:, :, t * P : (t + 1) * P]
                        )
                        xt = ph4.tile([P, D], F32, tag="xt4")
                        nc.sync.dma_start(xt[:], x_own[t * P : (t + 1) * P, :])
                        for n in range(2):
                            pp = psC.tile([P, 512], F32, tag="pj")
                            nc.tensor.matmul(
                                pp[:], ones_r[:, :P],
                                bproj_sb[:, n * 512 : (n + 1) * 512],
                                start=True, stop=False,
                            )
                            for ko in range(8):
                                nc.tensor.matmul(
                                    pp[:], yt_own[:, ko, :],
                                    wproj_sb[:, ko, n * 512 : (n + 1) * 512],
                                    start=False, stop=(ko == 7),
                                )
                            nc.vector.tensor_tensor(
                                x1_sb[:, t, n * 512 : (n + 1) * 512], pp[:],
                                xt[:, n * 512 : (n + 1) * 512], op=Alu.add,
                            )
                        layernorm_tile(
                            ph4, x1_sb[:, t, :], ln2w_sb, ln2b_sb, psD, h2T, t
                        )

                # =====================================================
                # Phase 5: MLP (own tokens, 2 groups of 512, bf16)
                # =====================================================
                with (
                    tc.tile_pool(name="mt", bufs=1) as mtp,
                    tc.tile_pool(name="w2p", bufs=1) as w2p,
                    tc.tile_pool(name="ph5", bufs=3) as ph5,
                    tc.tile_pool(name="psM", bufs=2, space="PSUM") as psM,
                    tc.tile_pool(name="psO", bufs=1, space="PSUM") as psO,
                    nc.named_scope("ph5_mlp"),
                ):
                    wfcv = wfc.rearrange("(ko p) n -> p ko n", p=P)
                    # w_fc2 fully resident in SBUF (bf16, 64KB/partition)
                    wfc2_sb = w2p.tile([P, DFF // P, D], BF16)
                    nc.sync.dma_start(
                        wfc2_sb[:], wfc2.rearrange("(kb p) n -> p kb n", p=P)
                    )
                    for g in range(2):
                        mT = mtp.tile([P, DFF // P, 512], BF16, tag="mt")
                        for kb in range(DFF // P):
                            wt = ph5.tile([P, 8, P], BF16, tag="wfct")
                            nc.sync.dma_start(
                                wt[:], wfcv[:, :, kb * P : (kb + 1) * P]
                            )
                            pm = psM.tile([P, 512], F32, tag="pm")
                            for ko in range(8):
                                nc.tensor.matmul(
                                    pm[:], wt[:, ko, :],
                                    h2T[:, ko, 4 * g : 4 * g + 4, :],
                                    start=(ko == 0), stop=(ko == 7),
                                )
                            nc.scalar.activation(
                                mT[:, kb, :], pm[:], Act.Gelu,
                                bias=bfc_sb[:, kb : kb + 1],
                            )
                        for n2 in range(2):
                            pos = [
                                psO.tile([P, 512], F32, tag=f"po{t2}",
                                         name=f"po_{g}_{n2}_{t2}")
                                for t2 in range(4)
                            ]
                            for t2 in range(4):
                                nc.tensor.matmul(
                                    pos[t2][:], ones_b[:, :P],
                                    bfc2_sb[:, n2 * 512 : (n2 + 1) * 512],
                                    start=True, stop=False,
                                )
                            for kb in range(DFF // P):
                                for t2 in range(4):
                                    nc.tensor.matmul(
                                        pos[t2][:], mT[:, kb, t2 * P : (t2 + 1) * P],
                                        wfc2_sb[:, kb, n2 * 512 : (n2 + 1) * 512],
                                        start=False, stop=(kb == DFF // P - 1),
                                    )
                            for t2 in range(4):
                                t = 4 * g + t2
                                ot = ph5.tile([P, 512], F32, tag="ot")
                                nc.vector.tensor_tensor(
                                    ot[:], pos[t2][:],
                                    x1_sb[:, t, n2 * 512 : (n2 + 1) * 512],
                                    op=Alu.add,
                                )
                                nc.sync.dma_start(
                                    out_own[t * P : (t + 1) * P,
                                            n2 * 512 : (n2 + 1) * 512],
                                    ot[:],
                                )

    nc.compile()
    return nc


_ones_col = None


def ones_col_bf(cst, nc):
    """[P, 16] fp32 tile of ones (source for vn's bf16 ones-column)."""
    global _ones_col
    if _ones_col is None:
        oc = cst.tile([P, 16], F32, name="ones_col")
        nc.vector.memset(oc[:], 1.0)
        _ones_col = oc
    return _ones_col[:]


_NC_CACHE = None


def kernel(x, ln1_w, ln1_b, ln2_w, ln2_b, w_attn, b_attn, w_proj, b_proj,
           w_fc, b_fc, w_fc2, b_fc2):
    global _NC_CACHE, _ones_col
    if _NC_CACHE is None:
        _ones_col = None
        _NC_CACHE = build()
    nc = _NC_CACHE

    f = np.ascontiguousarray
    x = np.asarray(x, np.float32)
    xf = x.reshape(TOK, D)
    w_attn = np.asarray(w_attn, np.float32)
    b_attn = np.asarray(b_attn, np.float32)

    def col(v, c):  # [128, 1] bias slice
        return f(np.asarray(v, np.float32)[c * P : (c + 1) * P].reshape(P, 1))

    def strip(v):  # [1024] -> [128, 8] with [p, a] = v[a*128 + p]
        return f(np.asarray(v, np.float32).reshape(-1, P).T)

    def bf(v):
        return f(np.asarray(v, np.float32).astype(BF))

    in_maps = []
    for c in range(NCORES):
        in_maps.append({
            "x_own": f(xf[c * TOWN : (c + 1) * TOWN]),
            "wq": bf(w_attn[:, P * c : P * (c + 1)]),
            "wk": bf(w_attn[:, D + P * c : D + P * (c + 1)]),
            "wv": bf(w_attn[:, 2 * D + P * c : 2 * D + P * (c + 1)]),
            "bq": col(b_attn, c),
            "bk": col(b_attn[D:], c),
            "bv": col(b_attn[2 * D:], c),
            "ln1w": strip(ln1_w), "ln1b": strip(ln1_b),
            "ln2w": strip(ln2_w), "ln2b": strip(ln2_b),
            "wproj": f(np.asarray(w_proj, np.float32)),
            "bproj": f(np.asarray(b_proj, np.float32).reshape(1, D)),
            "wfc": bf(w_fc),
            "bfc": strip(b_fc),
            "wfc2": bf(w_fc2),
            "bfc2": bf(np.asarray(b_fc2, np.float32).reshape(1, D)),
        })

    global _last_in_maps
    _last_in_maps = in_maps
    res = run_bass_kernel_spmd(nc, in_maps, core_ids=list(range(NCORES)))
    out = np.concatenate([res.results[c]["out"] for c in range(NCORES)], axis=0)
    return out.reshape(B, T, D)


_last_in_maps = None


# revision 20
# speedup vs baseline: 1.2063x; 1.0158x over previous
"""Trainium2 Bass kernel for a GPT-style transformer block.

Reference computation (B=4, T=2048, d=1024, 16 heads, dff=4096, fp32):
    h  = LN1(x);  qkv = h @ w_attn + b_attn
    y  = causal_attention(q, k, v);  x1 = x + y @ w_proj + b_proj
    h2 = LN2(x1); out = x1 + gelu(h2 @ w_fc + b_fc) @ w_fc2 + b_fc2

Sharding over 8 NeuronCores (one trn2 chip):
  - Attention is head-parallel: core c owns heads (2c, 2c+1). Each core
    computes LN1 for its own 1024-token slice, the per-core h^T shards are
    AllGathered (bf16), each core computes q/k/v (transposed layout) for
    its two heads over all 8192 tokens, and runs causal attention for its
    8 (batch, head) units. The per-core y^T shards are redistributed with
    an AllToAll into token-sharded form.
  - Everything after attention is token-parallel: core c owns flattened
    tokens [1024c, 1024c+1024) and computes proj+residual+LN2+MLP+residual
    for them with full-width weights.

Precision: the residual stream, layernorm statistics, attention scores
(QK in float32r = full-rate fp32) and softmax sums stay fp32-ish; the
h-shard, v/attention-weights and the MLP run in bf16 (all accumulation in
fp32 PSUM). Measured end-to-end relative error ~1e-3 against the fp32
reference. Attention uses the transposed-score layout: scores^T[kv, tq]
so the softmax denominator comes free from an appended ones-column in v,
and the causal mask is added (additive -1e9) to PSUM before the exp.
"""

import os
import sys

import numpy as np
import ml_dtypes

sys.path.insert(0, "/opt/trn_rl_repo")

import concourse.bass as bass  # noqa: E402
import concourse.mybir as mybir  # noqa: E402
import concourse.tile as tile  # noqa: E402
from concourse import bacc  # noqa: E402
from concourse.bass_utils import run_bass_kernel_spmd  # noqa: E402
from concourse.masks import make_identity  # noqa: E402

B, T, D, H, HD, DFF = 4, 2048, 1024, 16, 64, 4096
EPS = 1e-5
NCORES = 8
TOK = B * T            # 8192 flattened tokens
TOWN = TOK // NCORES   # 1024 tokens owned per core
P = 128
F32 = mybir.dt.float32
F32R = mybir.dt.float32r
BF16 = mybir.dt.bfloat16
Act = mybir.ActivationFunctionType
Alu = mybir.AluOpType
AX = mybir.AxisListType
NEG = -1.0e9
BF = ml_dtypes.bfloat16


def build():
    nc = bacc.Bacc("TRN2", target_bir_lowering=False, debug=False, num_devices=NCORES)

    def inp(name, shape, dt=F32):
        return nc.dram_tensor(name, shape, dt, kind="ExternalInput").ap()

    x_own = inp("x_own", [TOWN, D])
    wq = inp("wq", [D, P], BF16)
    wk = inp("wk", [D, P], BF16)
    wv = inp("wv", [D, P], BF16)
    bq = inp("bq", [P, 1])
    bk = inp("bk", [P, 1])
    bv = inp("bv", [P, 1])
    ln1w = inp("ln1w", [P, 8])
    ln1b = inp("ln1b", [P, 8])
    ln2w = inp("ln2w", [P, 8])
    ln2b = inp("ln2b", [P, 8])
    wproj = inp("wproj", [D, D], F32R)
    bproj = inp("bproj", [1, D], F32R)
    wfc = inp("wfc", [D, DFF], BF16)
    bfc = inp("bfc", [P, DFF // P])
    wfc2 = inp("wfc2", [DFF, D], BF16)
    bfc2 = inp("bfc2", [1, D], BF16)
    out_own = nc.dram_tensor("out", [TOWN, D], F32, kind="ExternalOutput").ap()

    groups = [list(range(NCORES))]

    with tile.TileContext(nc) as tc:
        with (
            tc.tile_pool(name="const", bufs=1) as cst,
            tc.tile_pool(name="dram", bufs=1, space="DRAM") as dram,
        ):
            # ---------------- constants ----------------
            ident = cst.tile([P, P], F32)
            make_identity(nc, ident)
            ident_bf = cst.tile([P, P], BF16)
            make_identity(nc, ident_bf)
            ones_f = cst.tile([1, P], F32)
            nc.vector.memset(ones_f[:], 1.0)
            ones_r = cst.tile([1, P], F32R)
            nc.scalar.copy(ones_r[:], ones_f[:])
            ones_b = cst.tile([1, P], BF16)
            nc.scalar.copy(ones_b[:], ones_f[:])
            ln1w_sb = cst.tile([P, 8], F32)
            nc.sync.dma_start(ln1w_sb[:], ln1w)
            ln1b_sb = cst.tile([P, 8], F32)
            nc.sync.dma_start(ln1b_sb[:], ln1b)
            ln2w_sb = cst.tile([P, 8], F32)
            nc.sync.dma_start(ln2w_sb[:], ln2w)
            ln2b_sb = cst.tile([P, 8], F32)
            nc.sync.dma_start(ln2b_sb[:], ln2b)
            bq_sb = cst.tile([P, 1], F32)
            nc.sync.dma_start(bq_sb[:], bq)
            bk_sb = cst.tile([P, 1], F32)
            nc.sync.dma_start(bk_sb[:], bk)
            bv_sb = cst.tile([P, 1], F32)
            nc.sync.dma_start(bv_sb[:], bv)
            bproj_sb = cst.tile([1, D], F32R)
            nc.sync.dma_start(bproj_sb[:], bproj)
            bfc_sb = cst.tile([P, DFF // P], F32)
            nc.sync.dma_start(bfc_sb[:], bfc)
            bfc2_sb = cst.tile([1, D], BF16)
            nc.sync.dma_start(bfc2_sb[:], bfc2)
            # additive causal masks for the 4 diagonal offsets:
            # maskadd[s][i, j] = 0 if i <= j - 128*s else -1e9
            # (accumulated into the score PSUM by an identity matmul)
            maskadd = cst.tile([P, 4, 512], F32)
            nc.vector.memset(maskadd[:], 0.0)
            for s in range(4):
                nc.gpsimd.affine_select(
                    out=maskadd[:, s, :],
                    in_=maskadd[:, s, :],
                    pattern=[[1, 512]],
                    channel_multiplier=-1,
                    base=-128 * s,
                    compare_op=Alu.is_ge,
                    fill=NEG,
                )
            maskadd_r = cst.tile([P, 4, 512], F32R)
            for s in range(4):
                nc.scalar.copy(maskadd_r[:, s, :], maskadd[:, s, :])
            ident_r = cst.tile([P, P], F32R)
            nc.scalar.copy(ident_r[:], ident[:])
            ones_col = cst.tile([P, 16], F32)
            nc.vector.memset(ones_col[:], 1.0)

            # DRAM intermediates: h^T shard halves (AllGathered, bf16) and
            # per-head y^T chunks (AllToAll: head-sharded -> token-sharded)
            hT_dram_a = dram.tile([D, TOWN // 2], BF16)
            hT_dram_b = dram.tile([D, TOWN // 2], BF16)
            hT_full_a = dram.tile([NCORES * D, TOWN // 2], BF16, addr_space="Shared")
            hT_full_b = dram.tile([NCORES * D, TOWN // 2], BF16, addr_space="Shared")
            yT_send = [dram.tile([NCORES, HD, TOWN], F32R, name=f"ys{h}")
                       for h in range(2)]
            yT_recv = [dram.tile([NCORES, HD, TOWN], F32R, name=f"yr{h}")
                       for h in range(2)]

            # =========================================================
            # Phase 1: LN1 over own tokens -> h^T shard -> AllGather
            # =========================================================
            def layernorm_tile(pool, xt, w_sb, b_sb, ps_pool, dstT, t):
                """LN a [128, D] token tile and write transposed blocks
                (with gamma/beta applied) into dstT[:, dblk, t, :] (bf16)."""
                ssum = pool.tile([P, 1], F32, tag="ssum")
                nc.vector.reduce_sum(ssum[:], xt[:], axis=AX.X)
                mean = pool.tile([P, 1], F32, tag="mean")
                nc.scalar.mul(mean[:], ssum[:], 1.0 / D)
                sq = pool.tile([P, D], F32, tag="sq")
                sumsq = pool.tile([P, 1], F32, tag="sumsq")
                nc.scalar.activation(sq[:], xt[:], Act.Square, accum_out=sumsq[:])
                msq = pool.tile([P, 1], F32, tag="msq")
                nc.vector.tensor_tensor(msq[:], mean[:], mean[:], op=Alu.mult)
                var = pool.tile([P, 1], F32, tag="var")
                nc.vector.tensor_scalar(var[:], sumsq[:], 1.0 / D, EPS, Alu.mult, Alu.add)
                nc.vector.tensor_tensor(var[:], var[:], msq[:], op=Alu.subtract)
                rinv = pool.tile([P, 1], F32, tag="rinv")
                nc.vector.reciprocal(rinv[:], var[:])
                rstd = pool.tile([P, 1], F32, tag="rstd")
                nc.scalar.sqrt(rstd[:], rinv[:])
                hh = pool.tile([P, D], F32, tag="hh")
                nc.vector.tensor_scalar(
                    hh[:], xt[:], mean[:], rstd[:], Alu.subtract, Alu.mult
                )
                for dblk in range(8):
                    pt = ps_pool.tile([P, P], F32, tag="lnt")
                    nc.tensor.transpose(pt[:], hh[:, dblk * P : (dblk + 1) * P], ident[:])
                    nc.scalar.activation(
                        dstT[:, dblk, t, :],
                        pt[:],
                        Act.Identity,
                        bias=b_sb[:, dblk : dblk + 1],
                        scale=w_sb[:, dblk : dblk + 1],
                    )

            with (
                tc.tile_pool(name="ph1", bufs=2) as ph1,
                tc.tile_pool(name="ph1T", bufs=1) as ph1T,
                tc.tile_pool(name="psA", bufs=2, space="PSUM") as psA,
                nc.named_scope("ph1_ln1"),
            ):
                hT_asm = ph1T.tile([P, 8, 8, P], BF16)  # [p, dblk, t, j]
                hTva = hT_dram_a.rearrange("(dblk p) t -> p dblk t", p=P)
                hTvb = hT_dram_b.rearrange("(dblk p) t -> p dblk t", p=P)
                for t in range(4):
                    xt = ph1.tile([P, D], F32, tag="xt")
                    nc.sync.dma_start(xt[:], x_own[t * P : (t + 1) * P, :])
                    layernorm_tile(ph1, xt, ln1w_sb, ln1b_sb, psA, hT_asm, t)
                for dblk in range(8):
                    nc.sync.dma_start(hTva[:, dblk, :], hT_asm[:, dblk, 0:4, :])
                nc.gpsimd.collective_compute(
                    "AllGather", Alu.bypass, replica_groups=groups,
                    ins=[hT_dram_a[:]], outs=[hT_full_a[:]],
                )
                for t in range(4, 8):
                    xt = ph1.tile([P, D], F32, tag="xt")
                    nc.sync.dma_start(xt[:], x_own[t * P : (t + 1) * P, :])
                    layernorm_tile(ph1, xt, ln1w_sb, ln1b_sb, psA, hT_asm, t)
                for dblk in range(8):
                    nc.sync.dma_start(hTvb[:, dblk, :], hT_asm[:, dblk, 4:8, :])
                nc.gpsimd.collective_compute(
                    "AllGather", Alu.bypass, replica_groups=groups,
                    ins=[hT_dram_b[:]], outs=[hT_full_b[:]],
                )

            # =========================================================
            # Phase 2: q^T/k^T (f32r) and v^T (bf16) for this core's two
            # heads over all tokens; token index = tile16 * 512 + j
            # =========================================================
            with tc.tile_pool(name="qkv", bufs=1) as qkvp:
                qT = qkvp.tile([P, 16, 512], F32R)
                kT = qkvp.tile([P, 16, 512], F32R)
                vT = qkvp.tile([P, 16, 512], BF16)
                with (
                    tc.tile_pool(name="wqkv", bufs=1) as wp,
                    tc.tile_pool(name="ph2", bufs=3) as ph2,
                    tc.tile_pool(name="psB", bufs=3, space="PSUM") as psB,
                    nc.named_scope("ph2_qkv"),
                ):
                    wq_sb = wp.tile([P, 8, P], BF16)
                    nc.sync.dma_start(wq_sb[:], wq.rearrange("(ko p) m -> p ko m", p=P))
                    wk_sb = wp.tile([P, 8, P], BF16)
                    nc.sync.dma_start(wk_sb[:], wk.rearrange("(ko p) m -> p ko m", p=P))
                    wv_sb = wp.tile([P, 8, P], BF16)
                    nc.sync.dma_start(wv_sb[:], wv.rearrange("(ko p) m -> p ko m", p=P))
                    hfva = hT_full_a.rearrange("(r ko p) t -> r p ko t", p=P, ko=8)
                    hfvb = hT_full_b.rearrange("(r ko p) t -> r p ko t", p=P, ko=8)
                    for n in range(2):
                        hfv = hfva if n == 0 else hfvb
                        for rr in range(8):
                            ht = ph2.tile([P, 8, 512], BF16, tag="ht")
                            nc.sync.dma_start(ht[:], hfv[rr])
                            for w_sb, b_sb, dstT in (
                                (wq_sb, bq_sb, qT),
                                (wk_sb, bk_sb, kT),
                                (wv_sb, bv_sb, vT),
                            ):
                                ps = psB.tile([P, 512], F32, tag="qkvps")
                                for ko in range(8):
                                    nc.tensor.matmul(
                                        ps[:], w_sb[:, ko, :], ht[:, ko, :],
                                        start=(ko == 0), stop=(ko == 7),
                                    )
                                nc.scalar.activation(
                                    dstT[:, rr * 2 + n, :], ps[:], Act.Identity,
                                    bias=b_sb[:],
                                )

                # =====================================================
                # Phase 3: causal attention for 8 (batch, head) units
                # =====================================================
                with (
                    tc.tile_pool(name="ph3", bufs=4) as ph3,
                    tc.tile_pool(name="ph3s", bufs=3) as ph3s,
                    tc.tile_pool(name="vnp", bufs=2) as vnp,
                    tc.tile_pool(name="psQK", bufs=3, space="PSUM") as psQK,
                    tc.tile_pool(name="psY", bufs=2, space="PSUM") as psY,
                    tc.tile_pool(name="psT", bufs=2, space="PSUM") as psT,
                    nc.named_scope("ph3_attn"),
                ):
                    for h in range(2):
                        hp = h * HD  # partition offset of this head
                        for b in range(B):
                            # v natural [kv, hd] with an appended ones column
                            vn = vnp.tile([P, 16, HD + 1], BF16, tag="vn")
                            nc.vector.tensor_copy(vn[:, :, HD], ones_col[:])
                            for kq in range(4):  # 4 transposes -> 1 eviction
                                pt = psT.tile([P, 4 * HD], BF16, tag="vtp")
                                for k2 in range(4):
                                    kb = kq * 4 + k2
                                    nc.tensor.transpose(
                                        pt[:, k2 * HD : (k2 + 1) * HD],
                                        vT[hp : hp + HD, b * 4 + kb // 4,
                                           (kb % 4) * P : (kb % 4 + 1) * P],
                                        ident_bf[hp : hp + HD, hp : hp + HD],
                                    )
                                nc.vector.tensor_copy(
                                    vn[:, kq * 4 : (kq + 1) * 4, 0:HD], pt[:]
                                )
                            for tq in range(4):
                                nkv = 4 * (tq + 1)
                                py = psY.tile([HD + 1, 512], F32, tag="py")
                                pend = []  # pipelined AV emission, lag 2
                                for kb in range(nkv):
                                    ps = psQK.tile([P, 512], F32, tag="qk")
                                    diag = kb >= 4 * tq
                                    nc.tensor.matmul(
                                        ps[:],
                                        kT[hp : hp + HD, b * 4 + kb // 4,
                                           (kb % 4) * P : (kb % 4 + 1) * P],
                                        qT[hp : hp + HD, b * 4 + tq, :],
                                        start=True, stop=not diag,
                                    )
                                    if diag:  # accumulate additive causal mask
                                        nc.tensor.matmul(
                                            ps[:], ident_r[:],
                                            maskadd_r[:, kb - 4 * tq, :],
                                            start=False, stop=True,
                                        )
                                    ex = ph3.tile([P, 512], BF16, tag="ex")
                                    nc.scalar.activation(
                                        ex[:], ps[:], Act.Exp, scale=1.0 / np.sqrt(HD)
                                    )
                                    pend.append((kb, ex))
                                    if len(pend) > 2:  # keep two QKs ahead of AV
                                        k0, e0 = pend.pop(0)
                                        nc.tensor.matmul(
                                            py[:], vn[:, k0, :], e0[:],
                                            start=(k0 == 0), stop=False,
                                        )
                                while pend:
                                    k0, e0 = pend.pop(0)
                                    nc.tensor.matmul(
                                        py[:], vn[:, k0, :], e0[:],
                                        start=(k0 == 0), stop=(not pend),
                                    )
                                # normalize by the ones-column sums (row HD)
                                rec = ph3s.tile([1, 512], F32, tag="rec")
                                nc.vector.reciprocal(rec[:], py[HD : HD + 1, :])
                                pbc = ph3s.tile([HD, 512], F32, tag="pbc")
                                nc.gpsimd.partition_broadcast(pbc[:], rec[:])
                                yt = ph3s.tile([HD, 512], F32, tag="yt")
                                nc.vector.tensor_tensor(
                                    yt[:], py[0:HD, :], pbc[:], op=Alu.mult
                                )
                                # global token start of this tq tile
                                t0 = b * T + tq * 512
                                nc.sync.dma_start(
                                    yT_send[h][t0 // TOWN, :,
                                               t0 % TOWN : t0 % TOWN + 512],
                                    yt[:].bitcast(F32R),
                                )
                        # redistribute this head's y^T while the other
                        # head's attention (or phase 4) runs
                        with nc.named_scope(f"cc_a2a_y{h}"):
                            nc.gpsimd.collective_compute(
                                "AllToAll", Alu.bypass, replica_groups=groups,
                                ins=[yT_send[h][:]], outs=[yT_recv[h][:]],
                            )

            # =========================================================
            # Phase 4: proj + residual + LN2 (own tokens) -> h2^T (SBUF)
            # =========================================================
            with tc.tile_pool(name="keep", bufs=1) as keep:
                h2T = keep.tile([P, 8, 8, P], BF16)  # [p, dblk, t, j]
                x1_sb = keep.tile([P, 8, D], F32)    # [p, t, d]
                with (
                    tc.tile_pool(name="ph4", bufs=2) as ph4,
                    tc.tile_pool(name="wpp", bufs=1) as wpp,
                    tc.tile_pool(name="psC", bufs=3, space="PSUM") as psC,
                    tc.tile_pool(name="psD", bufs=2, space="PSUM") as psD,
                    nc.named_scope("ph4_proj_ln2"),
                ):
                    wproj_sb = wpp.tile([P, 8, D], F32R)
                    nc.sync.dma_start(
                        wproj_sb[:], wproj.rearrange("(ko p) n -> p ko n", p=P)
                    )
                    # yT_recv[h][ko, p, t]: d = 128*ko + 64*h + p
                    yfv0 = yT_recv[0].rearrange("ko p t -> p ko t")
                    yfv1 = yT_recv[1].rearrange("ko p t -> p ko t")
                    for t in range(8):
                        yt_own = ph4.tile([P, 8, P], F32R, tag="ytown")
                        nc.sync.dma_start(
                            yt_own[0:HD, :, :], yfv0[:, :, t * P : (t + 1) * P]
                        )
                        nc.sync.dma_start(
                            yt_own[HD:P, :, :], yfv1[:, :, t * P : (t + 1) * P]
                        )
                        xt = ph4.tile([P, D], F32, tag="xt4")
                        nc.sync.dma_start(xt[:], x_own[t * P : (t + 1) * P, :])
                        for n in range(2):
                            pp = psC.tile([P, 512], F32, tag="pj")
                            nc.tensor.matmul(
                                pp[:], ones_r[:, :P],
                                bproj_sb[:, n * 512 : (n + 1) * 512],
                                start=True, stop=False,
                            )
                            for ko in range(8):
                                nc.tensor.matmul(
                                    pp[:], yt_own[:, ko, :],
                                    wproj_sb[:, ko, n * 512 : (n + 1) * 512],
                                    start=False, stop=(ko == 7),
                                )
                            nc.vector.tensor_tensor(
                                x1_sb[:, t, n * 512 : (n + 1) * 512], pp[:],
                                xt[:, n * 512 : (n + 1) * 512], op=Alu.add,
                            )
                        layernorm_tile(
                            ph4, x1_sb[:, t, :], ln2w_sb, ln2b_sb, psD, h2T, t
                        )

                # =====================================================
                # Phase 5: MLP (own tokens, 2 groups of 512, bf16)
                # =====================================================
                with (
                    tc.tile_pool(name="mt", bufs=1) as mtp,
                    tc.tile_pool(name="w2p", bufs=1) as w2p,
                    tc.tile_pool(name="ph5", bufs=3) as ph5,
                    tc.tile_pool(name="psM", bufs=2, space="PSUM") as psM,
                    tc.tile_pool(name="psO", bufs=1, space="PSUM") as psO,
                    nc.named_scope("ph5_mlp"),
                ):
                    wfcv = wfc.rearrange("(ko p) n -> p ko n", p=P)
                    # w_fc2 fully resident in SBUF (bf16, 64KB/partition)
                    wfc2_sb = w2p.tile([P, DFF // P, D], BF16)
                    nc.sync.dma_start(
                        wfc2_sb[:], wfc2.rearrange("(kb p) n -> p kb n", p=P)
                    )
                    for g in range(2):
                        mT = mtp.tile([P, DFF // P, 512], BF16, tag="mt")
                        for kb in range(DFF // P):
                            wt = ph5.tile([P, 8, P], BF16, tag="wfct")
                            nc.sync.dma_start(
                                wt[:], wfcv[:, :, kb * P : (kb + 1) * P]
                            )
                            pm = psM.tile([P, 512], F32, tag="pm")
                            for ko in range(8):
                                nc.tensor.matmul(
                                    pm[:], wt[:, ko, :],
                                    h2T[:, ko, 4 * g : 4 * g + 4, :],
                                    start=(ko == 0), stop=(ko == 7),
                                )
                            nc.scalar.activation(
                                mT[:, kb, :], pm[:], Act.Gelu,
                                bias=bfc_sb[:, kb : kb + 1],
                            )
                        for n2 in range(2):
                            pos = [
                                psO.tile([P, 512], F32, tag=f"po{t2}",
                                         name=f"po_{g}_{n2}_{t2}")
                                for t2 in range(4)
                            ]
                            for t2 in range(4):
                                nc.tensor.matmul(
                                    pos[t2][:], ones_b[:, :P],
                                    bfc2_sb[:, n2 * 512 : (n2 + 1) * 512],
                                    start=True, stop=False,
                                )
                            for kb in range(DFF // P):
                                for t2 in range(4):
                                    nc.tensor.matmul(
                                        pos[t2][:], mT[:, kb, t2 * P : (t2 + 1) * P],
                                        wfc2_sb[:, kb, n2 * 512 : (n2 + 1) * 512],
                                        start=False, stop=(kb == DFF // P - 1),
                                    )
                            for t2 in range(4):
                                t = 4 * g + t2
                                ot = ph5.tile([P, 512], F32, tag="ot")
                                nc.vector.tensor_tensor(
                                    ot[:], pos[t2][:],
                                    x1_sb[:, t, n2 * 512 : (n2 + 1) * 512],
                                    op=Alu.add,
                                )
                                nc.sync.dma_start(
                                    out_own[t * P : (t + 1) * P,
                                            n2 * 512 : (n2 + 1) * 512],
                                    ot[:],
                                )

    nc.compile()
    return nc


_NC_CACHE = None


def kernel(x, ln1_w, ln1_b, ln2_w, ln2_b, w_attn, b_attn, w_proj, b_proj,
           w_fc, b_fc, w_fc2, b_fc2):
    global _NC_CACHE
    if _NC_CACHE is None:
        _NC_CACHE = build()
    nc = _NC_CACHE

    f = np.ascontiguousarray
    x = np.asarray(x, np.float32)
    xf = x.reshape(TOK, D)
    w_attn = np.asarray(w_attn, np.float32)
    b_attn = np.asarray(b_attn, np.float32)

    def col(v, c):  # [128, 1] bias slice
        return f(np.asarray(v, np.float32)[c * P : (c + 1) * P].reshape(P, 1))

    def strip(v):  # [1024] -> [128, 8] with [p, a] = v[a*128 + p]
        return f(np.asarray(v, np.float32).reshape(-1, P).T)

    def bf(v):
        return f(np.asarray(v, np.float32).astype(BF))

    in_maps = []
    for c in range(NCORES):
        in_maps.append({
            "x_own": f(xf[c * TOWN : (c + 1) * TOWN]),
            "wq": bf(w_attn[:, P * c : P * (c + 1)]),
            "wk": bf(w_attn[:, D + P * c : D + P * (c + 1)]),
            "wv": bf(w_attn[:, 2 * D + P * c : 2 * D + P * (c + 1)]),
            "bq": col(b_attn, c),
            "bk": col(b_attn[D:], c),
            "bv": col(b_attn[2 * D:], c),
            "ln1w": strip(ln1_w), "ln1b": strip(ln1_b),
            "ln2w": strip(ln2_w), "ln2b": strip(ln2_b),
            "wproj": f(np.asarray(w_proj, np.float32)),
            "bproj": f(np.asarray(b_proj, np.float32).reshape(1, D)),
            "wfc": bf(w_fc),
            "bfc": strip(b_fc),
            "wfc2": bf(w_fc2),
            "bfc2": bf(np.asarray(b_fc2, np.float32).reshape(1, D)),
        })

    global _last_in_maps
    _last_in_maps = in_maps
    res = run_bass_kernel_spmd(nc, in_maps, core_ids=list(range(NCORES)))
    out = np.concatenate([res.results[c]["out"] for c in range(NCORES)], axis=0)
    return out.reshape(B, T, D)


_last_in_maps = None


# revision 22
# speedup vs baseline: 1.5288x; 1.2674x over previous
"""Trainium2 Bass kernel for a GPT-style transformer block.

Reference computation (B=4, T=2048, d=1024, 16 heads, dff=4096, fp32):
    h  = LN1(x);  qkv = h @ w_attn + b_attn
    y  = causal_attention(q, k, v);  x1 = x + y @ w_proj + b_proj
    h2 = LN2(x1); out = x1 + gelu(h2 @ w_fc + b_fc) @ w_fc2 + b_fc2

Sharding over 8 NeuronCores (one trn2 chip):
  - Attention is head-parallel: core c owns heads (2c, 2c+1). Each core
    computes LN1 for its own 1024-token slice, the per-core h^T shards are
    AllGathered (bf16, split in two halves overlapped with compute), each
    core computes q/k/v for its two heads over all 8192 tokens, and runs
    causal attention for its 8 (batch, head) units. Each head's y^T shard
    is redistributed with an AllToAll into token-sharded form (the first
    overlaps the second head's attention).
  - Everything after attention is token-parallel: core c owns flattened
    tokens [1024c, 1024c+1024) and computes proj+residual+LN2+MLP+residual
    for them with full-width weights.

Precision: residual stream, layernorm statistics, attention scores and
softmax sums are fp32 (all matmul accumulation in fp32 PSUM); matmul
operands are bf16. Measured end-to-end relative error ~2e-3 against the
fp32 reference.

PE-shape notes (these dominated performance): every matmul keeps the full
128-partition contraction so the PE activity monitor doesn't clock-gate
the array — per-head q tiles are zero-padded to 128 rows instead of
running 64-row matmuls — and all PE operands are bf16 so fast-weight-load
hides the LDWEIGHTS. Attention uses the transposed-score layout:
scores^T[kv, tq] so the softmax denominator comes free from an appended
ones-column in v, and the causal mask is accumulated into the score PSUM
by an identity matmul before the exp.
"""

import os
import sys

import numpy as np
import ml_dtypes

sys.path.insert(0, "/opt/trn_rl_repo")

import concourse.bass as bass  # noqa: E402
import concourse.mybir as mybir  # noqa: E402
import concourse.tile as tile  # noqa: E402
from concourse import bacc  # noqa: E402
from concourse.bass_utils import run_bass_kernel_spmd  # noqa: E402
from concourse.masks import make_identity  # noqa: E402

B, T, D, H, HD, DFF = 4, 2048, 1024, 16, 64, 4096
EPS = 1e-5
NCORES = 8
TOK = B * T            # 8192 flattened tokens
TOWN = TOK // NCORES   # 1024 tokens owned per core
P = 128
F32 = mybir.dt.float32
BF16 = mybir.dt.bfloat16
Act = mybir.ActivationFunctionType
Alu = mybir.AluOpType
AX = mybir.AxisListType
NEG = -1.0e9
BF = ml_dtypes.bfloat16


def build():
    nc = bacc.Bacc("TRN2", target_bir_lowering=False, debug=False, num_devices=NCORES)

    def inp(name, shape, dt=F32):
        return nc.dram_tensor(name, shape, dt, kind="ExternalInput").ap()

    x_own = inp("x_own", [TOWN, D])
    wq = inp("wq", [D, P], BF16)
    wk = inp("wk", [D, P], BF16)
    wv = inp("wv", [D, P], BF16)
    bq = inp("bq", [P, 1])
    bk = inp("bk", [P, 1])
    bv = inp("bv", [P, 1])
    ln1w = inp("ln1w", [P, 8])
    ln1b = inp("ln1b", [P, 8])
    ln2w = inp("ln2w", [P, 8])
    ln2b = inp("ln2b", [P, 8])
    wproj = inp("wproj", [D, D], BF16)
    bproj = inp("bproj", [1, D], BF16)
    wfc = inp("wfc", [D, DFF], BF16)
    bfc = inp("bfc", [P, DFF // P])
    wfc2 = inp("wfc2", [DFF, D], BF16)
    bfc2 = inp("bfc2", [1, D], BF16)
    out_own = nc.dram_tensor("out", [TOWN, D], F32, kind="ExternalOutput").ap()

    groups = [list(range(NCORES))]

    with tile.TileContext(nc) as tc:
        with (
            tc.tile_pool(name="const", bufs=1) as cst,
            tc.tile_pool(name="dram", bufs=1, space="DRAM") as dram,
        ):
            # ---------------- constants ----------------
            ident = cst.tile([P, P], F32)
            make_identity(nc, ident)
            ident_bf = cst.tile([P, P], BF16)
            make_identity(nc, ident_bf)
            ones_f = cst.tile([1, P], F32)
            nc.vector.memset(ones_f[:], 1.0)
            ones_b = cst.tile([1, P], BF16)
            nc.scalar.copy(ones_b[:], ones_f[:])
            ones_col = cst.tile([P, 16, 2], F32)
            nc.vector.memset(ones_col[:], 1.0)
            # per-head partition selectors (1 on own 64 rows, else 0)
            sel0 = cst.tile([P, 1], F32)
            nc.vector.memset(sel0[:], 1.0)
            nc.vector.memset(sel0[HD:P, :], 0.0)
            sel1 = cst.tile([P, 1], F32)
            nc.vector.memset(sel1[:], 0.0)
            nc.vector.memset(sel1[HD:P, :], 1.0)
            ln1w_sb = cst.tile([P, 8], F32)
            nc.sync.dma_start(ln1w_sb[:], ln1w)
            ln1b_sb = cst.tile([P, 8], F32)
            nc.sync.dma_start(ln1b_sb[:], ln1b)
            ln2w_sb = cst.tile([P, 8], F32)
            nc.sync.dma_start(ln2w_sb[:], ln2w)
            ln2b_sb = cst.tile([P, 8], F32)
            nc.sync.dma_start(ln2b_sb[:], ln2b)
            bq_sb = cst.tile([P, 1], F32)
            nc.sync.dma_start(bq_sb[:], bq)
            bk_sb = cst.tile([P, 1], F32)
            nc.sync.dma_start(bk_sb[:], bk)
            bv_sb = cst.tile([P, 1], F32)
            nc.sync.dma_start(bv_sb[:], bv)
            # biases masked per head for the padded q tiles
            bq0 = cst.tile([P, 1], F32)
            nc.vector.tensor_tensor(bq0[:], bq_sb[:], sel0[:], op=Alu.mult)
            bq1 = cst.tile([P, 1], F32)
            nc.vector.tensor_tensor(bq1[:], bq_sb[:], sel1[:], op=Alu.mult)
            bproj_sb = cst.tile([1, D], BF16)
            nc.sync.dma_start(bproj_sb[:], bproj)
            bfc_sb = cst.tile([P, DFF // P], F32)
            nc.sync.dma_start(bfc_sb[:], bfc)
            bfc2_sb = cst.tile([1, D], BF16)
            nc.sync.dma_start(bfc2_sb[:], bfc2)
            # additive causal masks for the 4 diagonal offsets (bf16):
            # maskadd[s][i, j] = 0 if i <= j - 128*s else -1e9
            # (accumulated into the score PSUM by an identity matmul)
            maskf = cst.tile([P, 4, 512], F32)
            nc.vector.memset(maskf[:], 0.0)
            for s in range(4):
                nc.gpsimd.affine_select(
                    out=maskf[:, s, :],
                    in_=maskf[:, s, :],
                    pattern=[[1, 512]],
                    channel_multiplier=-1,
                    base=-128 * s,
                    compare_op=Alu.is_ge,
                    fill=NEG,
                )
            maskadd = cst.tile([P, 4, 512], BF16)
            nc.scalar.copy(maskadd[:], maskf[:])

            # DRAM intermediates: h^T shard halves (AllGathered, bf16) and
            # per-head y^T chunks (AllToAll: head-sharded -> token-sharded)
            hT_dram_a = dram.tile([D, TOWN // 2], BF16)
            hT_dram_b = dram.tile([D, TOWN // 2], BF16)
            hT_full_a = dram.tile([NCORES * D, TOWN // 2], BF16, addr_space="Shared")
            hT_full_b = dram.tile([NCORES * D, TOWN // 2], BF16, addr_space="Shared")
            yT_send = [dram.tile([NCORES, HD, TOWN], BF16, name=f"ys{h}")
                       for h in range(2)]
            yT_recv = [dram.tile([NCORES, HD, TOWN], BF16, name=f"yr{h}")
                       for h in range(2)]

            # =========================================================
            # Phase 1: LN1 over own tokens -> h^T shard -> AllGather x2
            # =========================================================
            def layernorm_tile(pool, xt, w_sb, b_sb, ps_pool, dstT, t):
                """LN a [128, D] token tile and write transposed blocks
                (with gamma/beta applied) into dstT[:, dblk, t, :] (bf16)."""
                ssum = pool.tile([P, 1], F32, tag="ssum")
                nc.vector.reduce_sum(ssum[:], xt[:], axis=AX.X)
                mean = pool.tile([P, 1], F32, tag="mean")
                nc.scalar.mul(mean[:], ssum[:], 1.0 / D)
                sq = pool.tile([P, D], F32, tag="sq")
                sumsq = pool.tile([P, 1], F32, tag="sumsq")
                nc.scalar.activation(sq[:], xt[:], Act.Square, accum_out=sumsq[:])
                msq = pool.tile([P, 1], F32, tag="msq")
                nc.vector.tensor_tensor(msq[:], mean[:], mean[:], op=Alu.mult)
                var = pool.tile([P, 1], F32, tag="var")
                nc.vector.tensor_scalar(var[:], sumsq[:], 1.0 / D, EPS, Alu.mult, Alu.add)
                nc.vector.tensor_tensor(var[:], var[:], msq[:], op=Alu.subtract)
                rinv = pool.tile([P, 1], F32, tag="rinv")
                nc.vector.reciprocal(rinv[:], var[:])
                rstd = pool.tile([P, 1], F32, tag="rstd")
                nc.scalar.sqrt(rstd[:], rinv[:])
                hh = pool.tile([P, D], F32, tag="hh")
                nc.vector.tensor_scalar(
                    hh[:], xt[:], mean[:], rstd[:], Alu.subtract, Alu.mult
                )
                for dblk in range(8):
                    pt = ps_pool.tile([P, P], F32, tag="lnt")
                    nc.tensor.transpose(pt[:], hh[:, dblk * P : (dblk + 1) * P], ident[:])
                    nc.scalar.activation(
                        dstT[:, dblk, t, :],
                        pt[:],
                        Act.Identity,
                        bias=b_sb[:, dblk : dblk + 1],
                        scale=w_sb[:, dblk : dblk + 1],
                    )

            with (
                tc.tile_pool(name="ph1", bufs=2) as ph1,
                tc.tile_pool(name="ph1T", bufs=1) as ph1T,
                tc.tile_pool(name="psA", bufs=2, space="PSUM") as psA,
                nc.named_scope("ph1_ln1"),
            ):
                hT_asm = ph1T.tile([P, 8, 8, P], BF16)  # [p, dblk, t, j]
                hTva = hT_dram_a.rearrange("(dblk p) t -> p dblk t", p=P)
                hTvb = hT_dram_b.rearrange("(dblk p) t -> p dblk t", p=P)
                for t in range(4):
                    xt = ph1.tile([P, D], F32, tag="xt")
                    nc.sync.dma_start(xt[:], x_own[t * P : (t + 1) * P, :])
                    layernorm_tile(ph1, xt, ln1w_sb, ln1b_sb, psA, hT_asm, t)
                for dblk in range(8):
                    nc.sync.dma_start(hTva[:, dblk, :], hT_asm[:, dblk, 0:4, :])
                nc.gpsimd.collective_compute(
                    "AllGather", Alu.bypass, replica_groups=groups,
                    ins=[hT_dram_a[:]], outs=[hT_full_a[:]],
                )
                for t in range(4, 8):
                    xt = ph1.tile([P, D], F32, tag="xt")
                    nc.sync.dma_start(xt[:], x_own[t * P : (t + 1) * P, :])
                    layernorm_tile(ph1, xt, ln1w_sb, ln1b_sb, psA, hT_asm, t)
                for dblk in range(8):
                    nc.sync.dma_start(hTvb[:, dblk, :], hT_asm[:, dblk, 4:8, :])
                nc.gpsimd.collective_compute(
                    "AllGather", Alu.bypass, replica_groups=groups,
                    ins=[hT_dram_b[:]], outs=[hT_full_b[:]],
                )

            # =========================================================
            # Phase 2: q^T (zero-padded per head), k^T, v^T (bf16) for
            # this core's two heads over all tokens.
            # token index = tile16 * 512 + j,  tile16 = rr*2 + n
            # =========================================================
            with tc.tile_pool(name="qkv", bufs=1) as qkvp:
                qTp = [qkvp.tile([P, 16, 512], BF16, name=f"qTp{h}") for h in range(2)]
                kT = qkvp.tile([P, 16, 512], BF16)
                vT = qkvp.tile([P, 16, 512], BF16)
                with (
                    tc.tile_pool(name="wqkv", bufs=1) as wp,
                    tc.tile_pool(name="ph2", bufs=3) as ph2,
                    tc.tile_pool(name="psB", bufs=3, space="PSUM") as psB,
                    nc.named_scope("ph2_qkv"),
                ):
                    wq_sb = wp.tile([P, 8, P], BF16)
                    nc.sync.dma_start(wq_sb[:], wq.rearrange("(ko p) m -> p ko m", p=P))
                    wk_sb = wp.tile([P, 8, P], BF16)
                    nc.sync.dma_start(wk_sb[:], wk.rearrange("(ko p) m -> p ko m", p=P))
                    wv_sb = wp.tile([P, 8, P], BF16)
                    nc.sync.dma_start(wv_sb[:], wv.rearrange("(ko p) m -> p ko m", p=P))
                    hfva = hT_full_a.rearrange("(r ko p) t -> r p ko t", p=P, ko=8)
                    hfvb = hT_full_b.rearrange("(r ko p) t -> r p ko t", p=P, ko=8)
                    for n in range(2):
                        hfv = hfva if n == 0 else hfvb
                        for rr in range(8):
                            ht = ph2.tile([P, 8, 512], BF16, tag="ht")
                            nc.sync.dma_start(ht[:], hfv[rr])
                            for wi, (w_sb, b_sb) in enumerate(
                                ((wq_sb, bq_sb), (wk_sb, bk_sb), (wv_sb, bv_sb))
                            ):
                                ps = psB.tile([P, 512], F32, tag="qkvps")
                                for ko in range(8):
                                    nc.tensor.matmul(
                                        ps[:], w_sb[:, ko, :], ht[:, ko, :],
                                        start=(ko == 0), stop=(ko == 7),
                                    )
                                ti = rr * 2 + n
                                if wi == 0:
                                    # padded per-head q: other head's rows = 0
                                    nc.scalar.activation(
                                        qTp[0][:, ti, :], ps[:], Act.Identity,
                                        bias=bq0[:], scale=sel0[:],
                                    )
                                    nc.scalar.activation(
                                        qTp[1][:, ti, :], ps[:], Act.Identity,
                                        bias=bq1[:], scale=sel1[:],
                                    )
                                else:
                                    dstT = kT if wi == 1 else vT
                                    nc.scalar.activation(
                                        dstT[:, ti, :], ps[:], Act.Identity,
                                        bias=b_sb[:],
                                    )

                # =====================================================
                # Phase 3: causal attention for 8 (batch, head) units
                # =====================================================
                with (
                    tc.tile_pool(name="ph3", bufs=4) as ph3,
                    tc.tile_pool(name="ph3s", bufs=3) as ph3s,
                    tc.tile_pool(name="vnp", bufs=2) as vnp,
                    tc.tile_pool(name="psQK", bufs=2, space="PSUM") as psQK,
                    tc.tile_pool(name="psY", bufs=2, space="PSUM") as psY,
                    tc.tile_pool(name="psT", bufs=2, space="PSUM") as psT,
                    nc.named_scope("ph3_attn"),
                ):
                    for h in range(2):
                        hp = h * HD
                        for b in range(B):
                            # joint v natural [kv, {v0,ones,pad}|{v1,ones,pad}]
                            # (padded to 128 columns per head for full-array AV)
                            vn = vnp.tile([P, 16, 2 * P], BF16, tag="vn")
                            nc.vector.memset(vn[:], 0.0)
                            nc.vector.tensor_copy(vn[:, :, HD], ones_col[:, :, 0])
                            nc.vector.tensor_copy(
                                vn[:, :, P + HD], ones_col[:, :, 0]
                            )
                            for kq in range(4):  # 4 joint transposes -> 1 evict
                                pt = psT.tile([P, 4, P], BF16, tag="vtp")
                                for k2 in range(4):
                                    kb = kq * 4 + k2
                                    nc.tensor.transpose(
                                        pt[:, k2, :],
                                        vT[:, b * 4 + kb // 4,
                                           (kb % 4) * P : (kb % 4 + 1) * P],
                                        ident_bf[:],
                                    )
                                # pt cols: [h0 dims 64 | h1 dims 64] ->
                                # vn cols [0:64] and [128:192]
                                nc.vector.tensor_copy(
                                    vn[:, kq * 4 : (kq + 1) * 4, :]
                                    .rearrange("p t (h c) -> p t h c", h=2)
                                    [:, :, :, 0:HD],
                                    pt[:].rearrange("p t (h c) -> p t h c", h=2),
                                )
                            for tq in range(4):
                                nkv = 4 * (tq + 1)
                                py = psY.tile([P, 512], F32, tag="py")
                                pend = []  # pipelined AV emission (pairs)
                                for u in range(nkv // 2):
                                    ps = psQK.tile([P, 2, 512], F32, tag="qk")
                                    for k2 in range(2):
                                        kb = 2 * u + k2
                                        diag = kb >= 4 * tq
                                        nc.tensor.matmul(
                                            ps[:, k2, :],
                                            kT[:, b * 4 + kb // 4,
                                               (kb % 4) * P : (kb % 4 + 1) * P],
                                            qTp[h][:, b * 4 + tq, :],
                                            start=True, stop=not diag,
                                        )
                                        if diag:  # accumulate causal mask
                                            nc.tensor.matmul(
                                                ps[:, k2, :], ident_bf[:],
                                                maskadd[:, kb - 4 * tq, :],
                                                start=False, stop=True,
                                            )
                                    ex = ph3.tile([P, 2, 512], BF16, tag="ex")
                                    nc.scalar.activation(
                                        ex[:], ps[:], Act.Exp, scale=1.0 / np.sqrt(HD)
                                    )
                                    pend.append((u, ex))
                                    if len(pend) > 1:  # one QK pair ahead
                                        u0, e0 = pend.pop(0)
                                        for k2 in range(2):
                                            kb = 2 * u0 + k2
                                            nc.tensor.matmul(
                                                py[:],
                                                vn[:, kb, h * P : (h + 1) * P],
                                                e0[:, k2, :],
                                                start=(kb == 0), stop=False,
                                            )
                                while pend:
                                    u0, e0 = pend.pop(0)
                                    for k2 in range(2):
                                        kb = 2 * u0 + k2
                                        nc.tensor.matmul(
                                            py[:],
                                            vn[:, kb, h * P : (h + 1) * P],
                                            e0[:, k2, :],
                                            start=(kb == 0),
                                            stop=(not pend and k2 == 1),
                                        )
                                # normalize by the ones-column sums (row HD)
                                rec = ph3s.tile([1, 512], F32, tag="rec")
                                nc.vector.reciprocal(rec[:], py[HD : HD + 1, :])
                                pbc = ph3s.tile([HD, 512], F32, tag="pbc")
                                nc.gpsimd.partition_broadcast(pbc[:], rec[:])
                                yt = ph3s.tile([HD, 512], BF16, tag="yt")
                                nc.vector.tensor_tensor(
                                    yt[:], py[0:HD, :], pbc[:], op=Alu.mult
                                )
                                # global token start of this tq tile
                                t0 = b * T + tq * 512
                                nc.sync.dma_start(
                                    yT_send[h][t0 // TOWN, :,
                                               t0 % TOWN : t0 % TOWN + 512],
                                    yt[:],
                                )
                        # redistribute this head's y^T while the other
                        # head's attention (or phase 4) runs
                        with nc.named_scope(f"cc_a2a_y{h}"):
                            nc.gpsimd.collective_compute(
                                "AllToAll", Alu.bypass, replica_groups=groups,
                                ins=[yT_send[h][:]], outs=[yT_recv[h][:]],
                            )

            # =========================================================
            # Phase 4: proj + residual + LN2 (own tokens) -> h2^T (SBUF)
            # =========================================================
            with tc.tile_pool(name="keep", bufs=1) as keep:
                h2T = keep.tile([P, 8, 8, P], BF16)  # [p, dblk, t, j]
                x1_sb = keep.tile([P, 8, D], F32)    # [p, t, d]
                with (
                    tc.tile_pool(name="ph4", bufs=2) as ph4,
                    tc.tile_pool(name="wpp", bufs=1) as wpp,
                    tc.tile_pool(name="psC", bufs=3, space="PSUM") as psC,
                    tc.tile_pool(name="psD", bufs=2, space="PSUM") as psD,
                    nc.named_scope("ph4_proj_ln2"),
                ):
                    wproj_sb = wpp.tile([P, 8, D], BF16)
                    nc.sync.dma_start(
                        wproj_sb[:], wproj.rearrange("(ko p) n -> p ko n", p=P)
                    )
                    # yT_recv[h][ko, p, t]: d = 128*ko + 64*h + p
                    yfv0 = yT_recv[0].rearrange("ko p t -> p ko t")
                    yfv1 = yT_recv[1].rearrange("ko p t -> p ko t")
                    for t in range(8):
                        yt_own = ph4.tile([P, 8, P], BF16, tag="ytown")
                        nc.sync.dma_start(
                            yt_own[0:HD, :, :], yfv0[:, :, t * P : (t + 1) * P]
                        )
                        nc.sync.dma_start(
                            yt_own[HD:P, :, :], yfv1[:, :, t * P : (t + 1) * P]
                        )
                        xt = ph4.tile([P, D], F32, tag="xt4")
                        nc.sync.dma_start(xt[:], x_own[t * P : (t + 1) * P, :])
                        for n in range(2):
                            pp = psC.tile([P, 512], F32, tag="pj")
                            nc.tensor.matmul(
                                pp[:], ones_b[:, :P],
                                bproj_sb[:, n * 512 : (n + 1) * 512],
                                start=True, stop=False,
                            )
                            for ko in range(8):
                                nc.tensor.matmul(
                                    pp[:], yt_own[:, ko, :],
                                    wproj_sb[:, ko, n * 512 : (n + 1) * 512],
                                    start=False, stop=(ko == 7),
                                )
                            nc.vector.tensor_tensor(
                                x1_sb[:, t, n * 512 : (n + 1) * 512], pp[:],
                                xt[:, n * 512 : (n + 1) * 512], op=Alu.add,
                            )
                        layernorm_tile(
                            ph4, x1_sb[:, t, :], ln2w_sb, ln2b_sb, psD, h2T, t
                        )

                # =====================================================
                # Phase 5: MLP (own tokens, 2 groups of 512, bf16)
                # =====================================================
                with (
                    tc.tile_pool(name="mt", bufs=1) as mtp,
                    tc.tile_pool(name="w2p", bufs=1) as w2p,
                    tc.tile_pool(name="ph5", bufs=3) as ph5,
                    tc.tile_pool(name="psM", bufs=2, space="PSUM") as psM,
                    tc.tile_pool(name="psO", bufs=1, space="PSUM") as psO,
                    nc.named_scope("ph5_mlp"),
                ):
                    wfcv = wfc.rearrange("(ko p) n -> p ko n", p=P)
                    # w_fc2 fully resident in SBUF (bf16, 64KB/partition)
                    wfc2_sb = w2p.tile([P, DFF // P, D], BF16)
                    nc.sync.dma_start(
                        wfc2_sb[:], wfc2.rearrange("(kb p) n -> p kb n", p=P)
                    )
                    for g in range(2):
                        mT = mtp.tile([P, DFF // P, 512], BF16, tag="mt")
                        for kb in range(DFF // P):
                            wt = ph5.tile([P, 8, P], BF16, tag="wfct")
                            nc.sync.dma_start(
                                wt[:], wfcv[:, :, kb * P : (kb + 1) * P]
                            )
                            pm = psM.tile([P, 512], F32, tag="pm")
                            for ko in range(8):
                                nc.tensor.matmul(
                                    pm[:], wt[:, ko, :],
                                    h2T[:, ko, 4 * g : 4 * g + 4, :],
                                    start=(ko == 0), stop=(ko == 7),
                                )
                            nc.scalar.activation(
                                mT[:, kb, :], pm[:], Act.Gelu,
                                bias=bfc_sb[:, kb : kb + 1],
                            )
                        for n2 in range(2):
                            pos = [
                                psO.tile([P, 512], F32, tag=f"po{t2}",
                                         name=f"po_{g}_{n2}_{t2}")
                                for t2 in range(4)
                            ]
                            for t2 in range(4):
                                nc.tensor.matmul(
                                    pos[t2][:], ones_b[:, :P],
                                    bfc2_sb[:, n2 * 512 : (n2 + 1) * 512],
                                    start=True, stop=False,
                                )
                            for kb in range(DFF // P):
                                for t2 in range(4):
                                    nc.tensor.matmul(
                                        pos[t2][:], mT[:, kb, t2 * P : (t2 + 1) * P],
                                        wfc2_sb[:, kb, n2 * 512 : (n2 + 1) * 512],
                                        start=False, stop=(kb == DFF // P - 1),
                                    )
                            for t2 in range(4):
                                t = 4 * g + t2
                                ot = ph5.tile([P, 512], F32, tag="ot")
                                nc.vector.tensor_tensor(
                                    ot[:], pos[t2][:],
                                    x1_sb[:, t, n2 * 512 : (n2 + 1) * 512],
                                    op=Alu.add,
                                )
                                nc.sync.dma_start(
                                    out_own[t * P : (t + 1) * P,
                                            n2 * 512 : (n2 + 1) * 512],
                                    ot[:],
                                )

    nc.compile()
    return nc


_NC_CACHE = None


def kernel(x, ln1_w, ln1_b, ln2_w, ln2_b, w_attn, b_attn, w_proj, b_proj,
           w_fc, b_fc, w_fc2, b_fc2):
    global _NC_CACHE
    if _NC_CACHE is None:
        _NC_CACHE = build()
    nc = _NC_CACHE

    f = np.ascontiguousarray
    x = np.asarray(x, np.float32)
    xf = x.reshape(TOK, D)
    w_attn = np.asarray(w_attn, np.float32)
    b_attn = np.asarray(b_attn, np.float32)

    def col(v, c):  # [128, 1] bias slice
        return f(np.asarray(v, np.float32)[c * P : (c + 1) * P].reshape(P, 1))

    def strip(v):  # [1024] -> [128, 8] with [p, a] = v[a*128 + p]
        return f(np.asarray(v, np.float32).reshape(-1, P).T)

    def bf(v):
        return f(np.asarray(v, np.float32).astype(BF))

    in_maps = []
    for c in range(NCORES):
        in_maps.append({
            "x_own": f(xf[c * TOWN : (c + 1) * TOWN]),
            "wq": bf(w_attn[:, P * c : P * (c + 1)]),
            "wk": bf(w_attn[:, D + P * c : D + P * (c + 1)]),
            "wv": bf(w_attn[:, 2 * D + P * c : 2 * D + P * (c + 1)]),
            "bq": col(b_attn, c),
            "bk": col(b_attn[D:], c),
            "bv": col(b_attn[2 * D:], c),
            "ln1w": strip(ln1_w), "ln1b": strip(ln1_b),
            "ln2w": strip(ln2_w), "ln2b": strip(ln2_b),
            "wproj": bf(w_proj),
            "bproj": bf(np.asarray(b_proj, np.float32).reshape(1, D)),
            "wfc": bf(w_fc),
            "bfc": strip(b_fc),
            "wfc2": bf(w_fc2),
            "bfc2": bf(np.asarray(b_fc2, np.float32).reshape(1, D)),
        })

    global _last_in_maps
    _last_in_maps = in_maps
    res = run_bass_kernel_spmd(nc, in_maps, core_ids=list(range(NCORES)))
    out = np.concatenate([res.results[c]["out"] for c in range(NCORES)], axis=0)
    return out.reshape(B, T, D)


_last_in_maps = None
